# revision 28
# baseline (speedup 1.0000x reference)
"""GQA kernel for trn2, 8 cores: DP over batch (2) x TP over kv-head groups (4).

Each core computes, for its (batch b, kv-group g):
  - qkv projection for its 4 q-heads + 1 kv-head (q pre-scaled by 1/sqrt(dk))
  - RoPE on q/k
  - full (non-causal) attention for the 4 q-heads vs its kv-head
  - partial out-projection with its 2048 rows of W_out
Host sums the 4 per-group partials per batch and adds bias.

All matmuls bf16 (full PE rate); accumulation fp32. Softmax denominators
are computed off the tensor engine: DVE pairwise adds + GpSimd running
sums + one [128,1]x[128,512] ones-matmul per (i,head), reciprocal via
reciprocal_approx_fast, broadcast via gpsimd. Scores/exp/PV are software
pipelined at key-chunk granularity per head-pair so ACT exp throughput
(~1.1us per [128,1024]) hides under PE matmul streams. PSUM drains are
plain ACT copies; normalization happens in SBUF afterwards so PSUM banks
recycle fast and the PE never waits on the softmax-denominator chain.

Self-contained: hardcodes all shapes. kernel(**inputs) -> np.ndarray.
"""

import math
from contextlib import ExitStack

import numpy as np
import ml_dtypes

import concourse.bass as bass
import concourse.bacc as bacc
import concourse.tile as tile
import concourse.mybir as mybir
from concourse.bass_utils import run_bass_kernel_spmd
from concourse.masks import make_identity

F32 = mybir.dt.float32
BF16 = mybir.dt.bfloat16
L = 2048          # sequence length
D = 2048          # d_model
DK = 128          # head dim (q/k)
DV = 512          # head dim (v)
NHQ = 4           # q heads per core
CQK = NHQ * DK + DK   # 640 qk projection cols per core
NI = 4            # query chunks of 512
NJ = 16           # key chunks of 128
NDCH = 16         # d_model chunks of 128

_NC_CACHE = {}


def build_nc():
    if "nc" in _NC_CACHE:
        return _NC_CACHE["nc"]
    nc = bacc.Bacc("TRN2", target_bir_lowering=False, debug=False)

    x_d = nc.dram_tensor("x", [L, D], BF16, kind="ExternalInput")
    wqk_d = nc.dram_tensor("wqk", [D, CQK], BF16, kind="ExternalInput")
    wv_d = nc.dram_tensor("wv", [D, DV], BF16, kind="ExternalInput")
    wo_d = nc.dram_tensor("wo", [NHQ * DV, D], BF16, kind="ExternalInput")
    cos_d = nc.dram_tensor("cost", [DK, L], F32, kind="ExternalInput")
    sin_d = nc.dram_tensor("sint", [DK, L], F32, kind="ExternalInput")
    out_d = nc.dram_tensor("out", [L, D], BF16, kind="ExternalOutput")

    EXP = mybir.ActivationFunctionType.Exp

    with ExitStack() as ctx:
        tc = ctx.enter_context(tile.TileContext(nc))
        persist = ctx.enter_context(tc.tile_pool(name="persist", bufs=1))

        ident = persist.tile([128, 128], BF16)
        make_identity(nc, ident)
        ones = persist.tile([128, 1], BF16)
        nc.vector.memset(ones, 1.0)

        qT = persist.tile([128, NHQ, L], BF16)      # [dk, h, pos]
        kT = persist.tile([128, L], BF16)           # [dk, pos]
        v_sb = persist.tile([128, NJ, DV], BF16)    # [key_in_chunk, key_chunk, e]
        wo_sb = persist.tile([128, NDCH, D], BF16)  # [e_in_chunk, e_chunk, dm]

        # ---------------- Phase B: x^T, qkv projection, rope ----------------
        with tc.tile_pool(name="pb1", bufs=1) as pb1, \
             tc.tile_pool(name="pb2", bufs=2) as pb2, \
             tc.tile_pool(name="psB", bufs=1, space="PSUM") as psB:
            cosT = pb1.tile([128, L], F32)
            sinT = pb1.tile([128, L], F32)
            wv_sb = pb1.tile([128, NDCH, DV], BF16)
            wqk_sb = pb1.tile([128, NDCH, CQK], BF16)

            # first x chunk on the sync queue (latency critical), big
            # prefetches on otherwise-idle engine queues
            xns = {}
            for lsub in range(4):
                xn = pb2.tile([128, D], BF16, tag="xn", bufs=4)
                nc.sync.dma_start(out=xn, in_=x_d.ap()[lsub * 128:(lsub + 1) * 128, :])
                xns[lsub] = xn
            # wqk rides the fast hardware DGE right behind the first x rows;
            # the rest go on the (slow, ~90GB/s) software DGE in need-order
            nc.sync.dma_start(
                out=wqk_sb, in_=wqk_d.ap().rearrange("(t p) c -> p t c", p=128))
            nc.gpsimd.dma_start(out=cosT, in_=cos_d.ap())
            nc.gpsimd.dma_start(out=sinT, in_=sin_d.ap())
            nc.gpsimd.dma_start(
                out=wv_sb, in_=wv_d.ap().rearrange("(t p) c -> p t c", p=128))
            nc.gpsimd.dma_start(
                out=wo_sb, in_=wo_d.ap().rearrange("(t p) c -> p t c", p=128))

            for i in range(NI):
                xT = pb1.tile([128, NDCH, 512], BF16, tag="xT")
                # transpose x rows for this 512-query chunk
                for lsub in range(4):
                    if i > 0:
                        xn = pb2.tile([128, D], BF16, tag="xn", bufs=4)
                        l0 = i * 512 + lsub * 128
                        nc.sync.dma_start(out=xn, in_=x_d.ap()[l0:l0 + 128, :])
                    else:
                        xn = xns[lsub]
                    for dgrp in range(4):
                        ps = psB.tile([128, 512], BF16, tag="tr", bufs=3)
                        for k in range(4):
                            dch = dgrp * 4 + k
                            nc.tensor.transpose(
                                ps[:, k * 128:(k + 1) * 128],
                                xn[:, dch * 128:(dch + 1) * 128], ident)
                        nc.vector.tensor_copy(
                            out=xT[:, dgrp * 4:dgrp * 4 + 4,
                                   lsub * 128:(lsub + 1) * 128],
                            in_=ps.rearrange("p (a b) -> p a b", a=4))

                # q/k projection + rope (c = 0..3 q heads, c = 4 is k)
                for c in range(5):
                    ps = psB.tile([128, 512], F32, tag="acc", bufs=3)
                    for t in range(NDCH):
                        nc.tensor.matmul(ps, lhsT=wqk_sb[:, t, c * 128:(c + 1) * 128],
                                         rhs=xT[:, t, :],
                                         start=(t == 0), stop=(t == NDCH - 1))
                    isl = slice(i * 512, (i + 1) * 512)
                    dest = qT[:, c, isl] if c < NHQ else kT[:, isl]
                    cs = cosT[:, isl]
                    sn = sinT[:, isl]
                    tmp = pb2.tile([128, 512], F32, tag="rope")
                    nc.vector.tensor_mul(tmp[0:64, :], ps[64:128, :], sn[0:64, :])
                    nc.vector.tensor_mul(tmp[64:128, :], ps[0:64, :], sn[64:128, :])
                    tmp2 = pb2.tile([128, 512], F32, tag="rope2")
                    nc.vector.tensor_mul(tmp2, ps, cs)
                    nc.vector.tensor_sub(dest[0:64, :], tmp2[0:64, :], tmp[0:64, :])
                    nc.vector.tensor_add(dest[64:128, :], tmp2[64:128, :],
                                         tmp[64:128, :])

                # v projection for these 4 key chunks
                for lsub in range(4):
                    ps = psB.tile([128, 512], F32, tag="acc", bufs=3)
                    for t in range(NDCH):
                        nc.tensor.matmul(
                            ps, lhsT=xT[:, t, lsub * 128:(lsub + 1) * 128],
                            rhs=wv_sb[:, t, :],
                            start=(t == 0), stop=(t == NDCH - 1))
                    nc.scalar.copy(out=v_sb[:, i * 4 + lsub, :], in_=ps)

        # ---------------- Phase C+D: attention + out-projection -------------
        with tc.tile_pool(name="pc1", bufs=1) as pc1, \
             tc.tile_pool(name="pc2", bufs=2) as pc2, \
             tc.tile_pool(name="psC", bufs=1, space="PSUM") as psC:
            for i in range(NI):
                isl = slice(i * 512, (i + 1) * 512)
                ctxT = {h: pc1.tile([128, 4, 512], BF16, tag=f"ctx{h}",
                                    name=f"ctxT{i}_{h}")
                        for h in range(NHQ)}

                for pair in range(2):
                    h0 = 2 * pair
                    # expS[keys, hh, j, q] for this head pair
                    expS = pc1.tile([128, 2, NJ, 512], BF16, tag="expS")
                    # denominator reduction tree over key chunks (j):
                    # L1/L2 bf16, L3 bf16, L4 fp32 per-lane totals
                    sacc = pc1.tile([128, 2, 8, 512], BF16, tag="sacc")
                    sac2 = pc1.tile([128, 2, 4, 512], BF16, tag="sac2")
                    sac3 = pc1.tile([128, 2, 2, 512], BF16, tag="sac3")
                    sac4 = pc1.tile([128, 2, 512], BF16, tag="sac4")
                    recipS = pc1.tile([1, 2, 512], F32, tag="recipS")
                    rb = pc1.tile([128, 2, 512], BF16, tag="rb")
                    ctxU = pc1.tile([128, 2, 2, 512], BF16, tag="ctxU")

                    pv1 = [psC.tile([128, 512], F32, tag="pv", bufs=4,
                                    name=f"pv1_{i}_{pair}_{n}") for n in range(4)]

                    def emit_pv(j, ecs, banks):
                        for ec in ecs:
                            for hh in range(2):
                                nc.tensor.matmul(
                                    banks[2 * (ec % 2) + hh],
                                    lhsT=v_sb[:, j, ec * 128:(ec + 1) * 128],
                                    rhs=expS[:, hh, j, :],
                                    start=(j == 0), stop=(j == NJ - 1))

                    def tree_add(dst, src, k, dt_note=None):
                        for hh in range(2):
                            nc.vector.tensor_add(out=dst[:, hh, k, :],
                                                 in0=src[:, hh, 2 * k, :],
                                                 in1=src[:, hh, 2 * k + 1, :])

                    # --- pass 1: scores/exp pipelined with denom + PV ec 0,1
                    for j in range(NJ):
                        jsl = slice(j * 128, (j + 1) * 128)
                        meg = psC.tile([128, 1024], F32, tag="ps", bufs=2,
                                       name=f"meg_{i}_{pair}_{j}")
                        nc.tensor.matmul(meg[:, 0:512], lhsT=kT[:, jsl],
                                         rhs=qT[:, h0, isl])
                        nc.tensor.matmul(meg[:, 512:1024], lhsT=kT[:, jsl],
                                         rhs=qT[:, h0 + 1, isl])
                        nc.scalar.activation(out=expS[:, :, j, :], in_=meg,
                                             func=EXP)
                        if j % 2 == 1:
                            tree_add(sacc, expS, j // 2)
                        if j % 4 == 3:
                            tree_add(sac2, sacc, j // 4)
                        if j % 8 == 7:
                            tree_add(sac3, sac2, j // 8)
                        if j >= 2:
                            emit_pv(j - 2, (0, 1), pv1)
                    emit_pv(NJ - 2, (0, 1), pv1)
                    emit_pv(NJ - 1, (0, 1), pv1)
                    # drain pass-1 banks (DVE copies; normalize later)
                    for ec in (0, 1):
                        for hh in range(2):
                            nc.vector.tensor_copy(out=ctxU[:, hh, ec, :],
                                                  in_=pv1[2 * ec + hh])

                    pv2 = [psC.tile([128, 512], F32, tag="pv", bufs=4,
                                    name=f"pv2_{i}_{pair}_{n}") for n in range(4)]
                    psos = []
                    for j in range(NJ):
                        emit_pv(j, (2, 3), pv2)
                        if j == 0:
                            # finish the per-lane tree (DVE)
                            for hh in range(2):
                                nc.vector.tensor_add(out=sac4[:, hh, :],
                                                     in0=sac3[:, hh, 0, :],
                                                     in1=sac3[:, hh, 1, :])
                        if j == 3:
                            # cross-partition sum via one ones-matmul per
                            # head, then recip + broadcast
                            for hh in range(2):
                                pso = psC.tile([1, 512], F32, tag="ps",
                                               bufs=2,
                                               name=f"pso_{i}_{pair}_{hh}")
                                nc.tensor.matmul(pso, lhsT=ones[:, 0:1],
                                                 rhs=sac4[:, hh, :])
                                psos.append(pso)
                            for hh in range(2):
                                nc.vector.reciprocal_approx_fast(
                                    out=recipS[:, hh, :], in_=psos[hh])
                            recipB = pc1.tile([1, 2, 512], BF16, tag="recipB")
                            nc.vector.tensor_copy(out=recipB, in_=recipS)
                            for hh in range(2):
                                nc.gpsimd.partition_broadcast(
                                    rb[:, hh, :], recipB[:, hh, :])
                        if j == 6:
                            # normalize pass-1 ctx (SBUF -> SBUF)
                            for ec in (0, 1):
                                for hh in range(2):
                                    nc.vector.tensor_mul(
                                        ctxT[h0 + hh][:, ec, :],
                                        ctxU[:, hh, ec, :],
                                        rb[:, hh, :])
                    # pass-2: normalize directly from PSUM
                    for ec in (2, 3):
                        for hh in range(2):
                            nc.vector.tensor_mul(
                                ctxT[h0 + hh][:, ec, :],
                                pv2[2 * (ec % 2) + hh],
                                rb[:, hh, :])

                # --- out-projection for this query chunk ---
                for dm in range(4):
                    for lsub in range(4):
                        ps = psC.tile([128, 512], F32, tag="pv", bufs=4,
                                      name=f"po_{i}_{dm}_{lsub}")
                        for t in range(NDCH):
                            nc.tensor.matmul(
                                ps,
                                lhsT=ctxT[t // 4][:, t % 4,
                                                  lsub * 128:(lsub + 1) * 128],
                                rhs=wo_sb[:, t, dm * 512:(dm + 1) * 512],
                                start=(t == 0), stop=(t == NDCH - 1))
                        ost = pc2.tile([128, 512], BF16, tag="ost")
                        nc.scalar.copy(out=ost, in_=ps)
                        l0 = i * 512 + lsub * 128
                        nc.sync.dma_start(
                            out=out_d.ap()[l0:l0 + 128,
                                           dm * 512:(dm + 1) * 512],
                            in_=ost)

    nc.compile()
    _NC_CACHE["nc"] = nc
    return nc


def make_core_inputs(x, W_attn, W_out):
    """Split full inputs into 8 per-core input maps (core = b*4 + g)."""
    Q_DIM = 2048
    K_DIM = 512
    scale = np.float32(1.0 / math.sqrt(DK))
    bf = ml_dtypes.bfloat16

    # rope tables, mirroring the fp32 reference computation
    inv_freq = (np.float32(1.0) /
                (np.float32(10000.0) **
                 (np.arange(0, DK, 2, dtype=np.float32) / np.float32(DK))))
    freqs = np.arange(L, dtype=np.float32)[:, None] * inv_freq[None, :]  # [L,64]
    ang = np.concatenate([freqs, freqs], axis=-1)  # [L, 128]
    cosT = np.ascontiguousarray(np.cos(ang).T.astype(np.float32))  # [128, L]
    sinT = np.ascontiguousarray(np.sin(ang).T.astype(np.float32))

    in_maps = []
    for core in range(8):
        b, g = divmod(core, 4)
        wq = (W_attn[:, 512 * g:512 * (g + 1)] * scale)
        wk = W_attn[:, Q_DIM + 128 * g:Q_DIM + 128 * (g + 1)]
        wqk = np.ascontiguousarray(
            np.concatenate([wq, wk], axis=1)).astype(bf)
        wv = np.ascontiguousarray(W_attn[:, Q_DIM + K_DIM + 512 * g:
                                         Q_DIM + K_DIM + 512 * (g + 1)]).astype(bf)
        wo = np.ascontiguousarray(W_out[2048 * g:2048 * (g + 1), :]).astype(bf)
        in_maps.append({
            "x": np.ascontiguousarray(x[b]).astype(bf),
            "wqk": wqk,
            "wv": wv,
            "wo": wo,
            "cost": cosT,
            "sint": sinT,
        })
    return in_maps


def kernel(x, W_attn, W_out, b_out, _trace=False, _trace_cores=None):
    x = np.asarray(x)
    W_attn = np.asarray(W_attn)
    W_out = np.asarray(W_out)
    b_out = np.asarray(b_out)
    nc = build_nc()
    in_maps = make_core_inputs(x, W_attn, W_out)
    res = run_bass_kernel_spmd(
        nc, in_maps, core_ids=list(range(8)),
        trace=_trace, trace_cores=_trace_cores)
    parts = [res.results[c]["out"] for c in range(8)]
    out = np.empty((2, L, D), dtype=np.float32)
    for b in range(2):
        acc = parts[4 * b].astype(np.float32)
        for g in range(1, 4):
            acc = acc + parts[4 * b + g].astype(np.float32)
        out[b] = acc + b_out[None, :].astype(np.float32)
    if _trace:
        kernel._last_results = res
    return out


# revision 34
# speedup vs baseline: 1.0539x; 1.0539x over previous
"""GQA kernel for trn2, 8 cores: DP over batch (2) x TP over kv-head groups (4).

Each core computes, for its (batch b, kv-group g):
  - qkv projection for its 4 q-heads + 1 kv-head (q pre-scaled by 1/sqrt(dk))
  - RoPE on q/k
  - full (non-causal) attention for the 4 q-heads vs its kv-head
  - partial out-projection with its 2048 rows of W_out
Host sums the 4 per-group partials per batch and adds bias.

All matmuls bf16 (full PE rate); accumulation fp32. Softmax denominators
are computed off the tensor engine: DVE pairwise adds + GpSimd running
sums + one [128,1]x[128,512] ones-matmul per (i,head), reciprocal via
reciprocal_approx_fast, broadcast via gpsimd. Scores/exp/PV are software
pipelined at key-chunk granularity per head-pair so ACT exp throughput
(~1.1us per [128,1024]) hides under PE matmul streams. PSUM drains are
plain ACT copies; normalization happens in SBUF afterwards so PSUM banks
recycle fast and the PE never waits on the softmax-denominator chain.

Self-contained: hardcodes all shapes. kernel(**inputs) -> np.ndarray.
"""

import math
from contextlib import ExitStack

import numpy as np
import ml_dtypes

import concourse.bass as bass
import concourse.bacc as bacc
import concourse.tile as tile
import concourse.mybir as mybir
from concourse.bass_utils import run_bass_kernel_spmd
from concourse.masks import make_identity

F32 = mybir.dt.float32
BF16 = mybir.dt.bfloat16
L = 2048          # sequence length
D = 2048          # d_model
DK = 128          # head dim (q/k)
DV = 512          # head dim (v)
NHQ = 4           # q heads per core
CQK = NHQ * DK + DK   # 640 qk projection cols per core
NI = 4            # query chunks of 512
NJ = 16           # key chunks of 128
NDCH = 16         # d_model chunks of 128

_NC_CACHE = {}


def build_nc():
    if "nc" in _NC_CACHE:
        return _NC_CACHE["nc"]
    nc = bacc.Bacc("TRN2", target_bir_lowering=False, debug=False)

    # weights arrive pre-rearranged from the host as [p, t, c] blocks so
    # every load is a plain contiguous copy on the hardware DGE
    x_d = nc.dram_tensor("x", [L, D], BF16, kind="ExternalInput")
    wqk_d = nc.dram_tensor("wqk", [128, NDCH, CQK], BF16, kind="ExternalInput")
    wv_d = nc.dram_tensor("wv", [128, NDCH, DV], BF16, kind="ExternalInput")
    wo_d = nc.dram_tensor("wo", [128, NDCH, D], BF16, kind="ExternalInput")
    cos_d = nc.dram_tensor("cost", [DK, L], F32, kind="ExternalInput")
    sin_d = nc.dram_tensor("sint", [DK, L], F32, kind="ExternalInput")
    out_d = nc.dram_tensor("out", [L, D], BF16, kind="ExternalOutput")

    EXP = mybir.ActivationFunctionType.Exp

    with ExitStack() as ctx:
        tc = ctx.enter_context(tile.TileContext(nc))
        persist = ctx.enter_context(tc.tile_pool(name="persist", bufs=1))

        ident = persist.tile([128, 128], BF16)
        make_identity(nc, ident)
        ones = persist.tile([128, 1], BF16)
        nc.vector.memset(ones, 1.0)

        qT = persist.tile([128, NHQ, L], BF16)      # [dk, h, pos]
        kT = persist.tile([128, L], BF16)           # [dk, pos]
        v_sb = persist.tile([128, NJ, DV], BF16)    # [key_in_chunk, key_chunk, e]
        wo_sb = persist.tile([128, NDCH, D], BF16)  # [e_in_chunk, e_chunk, dm]

        # ---------------- Phase B: x^T, qkv projection, rope ----------------
        with tc.tile_pool(name="pb1", bufs=1) as pb1, \
             tc.tile_pool(name="pb2", bufs=2) as pb2, \
             tc.tile_pool(name="psB", bufs=1, space="PSUM") as psB:
            cosT = pb1.tile([128, L], F32)
            sinT = pb1.tile([128, L], F32)
            wv_sb = pb1.tile([128, NDCH, DV], BF16)
            wqk_sb = pb1.tile([128, NDCH, CQK], BF16)

            # first x chunk on the sync queue (latency critical), big
            # prefetches on otherwise-idle engine queues
            xns = {}
            for lsub in range(4):
                xn = pb2.tile([128, D], BF16, tag="xn", bufs=4)
                nc.sync.dma_start(out=xn, in_=x_d.ap()[lsub * 128:(lsub + 1) * 128, :])
                xns[lsub] = xn
            # everything rides the fast hardware DGE in need-order; wv/wo
            # are issued later in program order so they don't block x rows
            nc.sync.dma_start(out=wqk_sb, in_=wqk_d.ap())
            nc.sync.dma_start(out=cosT, in_=cos_d.ap())
            nc.sync.dma_start(out=sinT, in_=sin_d.ap())

            for i in range(NI):
                xT = pb1.tile([128, NDCH, 512], BF16, tag="xT")
                # transpose x rows for this 512-query chunk
                for lsub in range(4):
                    if i > 0:
                        xn = pb2.tile([128, D], BF16, tag="xn", bufs=4)
                        l0 = i * 512 + lsub * 128
                        nc.sync.dma_start(out=xn, in_=x_d.ap()[l0:l0 + 128, :])
                    else:
                        xn = xns[lsub]
                    if i == 0 and lsub == 1:
                        nc.sync.dma_start(out=wv_sb, in_=wv_d.ap())
                    if i == 2 and lsub == 0:
                        nc.sync.dma_start(out=wo_sb, in_=wo_d.ap())
                    for dgrp in range(4):
                        ps = psB.tile([128, 512], BF16, tag="tr", bufs=3)
                        for k in range(4):
                            dch = dgrp * 4 + k
                            nc.tensor.transpose(
                                ps[:, k * 128:(k + 1) * 128],
                                xn[:, dch * 128:(dch + 1) * 128], ident)
                        nc.vector.tensor_copy(
                            out=xT[:, dgrp * 4:dgrp * 4 + 4,
                                   lsub * 128:(lsub + 1) * 128],
                            in_=ps.rearrange("p (a b) -> p a b", a=4))

                # q/k projection + rope (c = 0..3 q heads, c = 4 is k)
                for c in range(5):
                    ps = psB.tile([128, 512], F32, tag="acc", bufs=3)
                    for t in range(NDCH):
                        nc.tensor.matmul(ps, lhsT=wqk_sb[:, t, c * 128:(c + 1) * 128],
                                         rhs=xT[:, t, :],
                                         start=(t == 0), stop=(t == NDCH - 1))
                    isl = slice(i * 512, (i + 1) * 512)
                    dest = qT[:, c, isl] if c < NHQ else kT[:, isl]
                    cs = cosT[:, isl]
                    sn = sinT[:, isl]
                    tmp = pb2.tile([128, 512], F32, tag="rope")
                    nc.vector.tensor_mul(tmp[0:64, :], ps[64:128, :], sn[0:64, :])
                    nc.vector.tensor_mul(tmp[64:128, :], ps[0:64, :], sn[64:128, :])
                    tmp2 = pb2.tile([128, 512], F32, tag="rope2")
                    nc.vector.tensor_mul(tmp2, ps, cs)
                    nc.vector.tensor_sub(dest[0:64, :], tmp2[0:64, :], tmp[0:64, :])
                    nc.vector.tensor_add(dest[64:128, :], tmp2[64:128, :],
                                         tmp[64:128, :])

                # v projection for these 4 key chunks
                for lsub in range(4):
                    ps = psB.tile([128, 512], F32, tag="acc", bufs=3)
                    for t in range(NDCH):
                        nc.tensor.matmul(
                            ps, lhsT=xT[:, t, lsub * 128:(lsub + 1) * 128],
                            rhs=wv_sb[:, t, :],
                            start=(t == 0), stop=(t == NDCH - 1))
                    nc.scalar.copy(out=v_sb[:, i * 4 + lsub, :], in_=ps)

        # ---------------- Phase C+D: attention + out-projection -------------
        with tc.tile_pool(name="pc1", bufs=1) as pc1, \
             tc.tile_pool(name="pc2", bufs=2) as pc2, \
             tc.tile_pool(name="psC", bufs=1, space="PSUM") as psC:
            for i in range(NI):
                isl = slice(i * 512, (i + 1) * 512)
                ctxT = {h: pc1.tile([128, 4, 512], BF16, tag=f"ctx{h}",
                                    name=f"ctxT{i}_{h}")
                        for h in range(NHQ)}

                for pair in range(2):
                    h0 = 2 * pair
                    # expS[keys, hh, j, q] for this head pair
                    expS = pc1.tile([128, 2, NJ, 512], BF16, tag="expS")
                    # denominator reduction tree over key chunks (j):
                    # L1/L2 bf16, L3 bf16, L4 fp32 per-lane totals
                    sacc = pc1.tile([128, 2, 8, 512], BF16, tag="sacc")
                    sac2 = pc1.tile([128, 2, 4, 512], BF16, tag="sac2")
                    sac3 = pc1.tile([128, 2, 2, 512], BF16, tag="sac3")
                    sac4 = pc1.tile([128, 2, 512], BF16, tag="sac4")
                    recipS = pc1.tile([1, 2, 512], F32, tag="recipS")
                    rb = pc1.tile([128, 2, 512], BF16, tag="rb")
                    ctxU = pc1.tile([128, 2, 2, 512], BF16, tag="ctxU")

                    pv1 = [psC.tile([128, 512], F32, tag="pv", bufs=4,
                                    name=f"pv1_{i}_{pair}_{n}") for n in range(4)]

                    def emit_pv(j, ecs, banks):
                        for ec in ecs:
                            for hh in range(2):
                                nc.tensor.matmul(
                                    banks[2 * (ec % 2) + hh],
                                    lhsT=v_sb[:, j, ec * 128:(ec + 1) * 128],
                                    rhs=expS[:, hh, j, :],
                                    start=(j == 0), stop=(j == NJ - 1))

                    def tree_add(dst, src, k, dt_note=None):
                        for hh in range(2):
                            nc.vector.tensor_add(out=dst[:, hh, k, :],
                                                 in0=src[:, hh, 2 * k, :],
                                                 in1=src[:, hh, 2 * k + 1, :])

                    # --- pass 1: scores/exp pipelined with denom + PV ec 0,1
                    for j in range(NJ):
                        jsl = slice(j * 128, (j + 1) * 128)
                        meg = psC.tile([128, 1024], F32, tag="ps", bufs=2,
                                       name=f"meg_{i}_{pair}_{j}")
                        nc.tensor.matmul(meg[:, 0:512], lhsT=kT[:, jsl],
                                         rhs=qT[:, h0, isl])
                        nc.tensor.matmul(meg[:, 512:1024], lhsT=kT[:, jsl],
                                         rhs=qT[:, h0 + 1, isl])
                        nc.scalar.activation(out=expS[:, :, j, :], in_=meg,
                                             func=EXP)
                        if j % 2 == 1:
                            tree_add(sacc, expS, j // 2)
                        if j % 4 == 3:
                            tree_add(sac2, sacc, j // 4)
                        if j % 8 == 7:
                            tree_add(sac3, sac2, j // 8)
                        if j >= 2:
                            emit_pv(j - 2, (0, 1), pv1)
                    emit_pv(NJ - 2, (0, 1), pv1)
                    emit_pv(NJ - 1, (0, 1), pv1)
                    # drain pass-1 banks (DVE copies; normalize later)
                    for ec in (0, 1):
                        for hh in range(2):
                            nc.vector.tensor_copy(out=ctxU[:, hh, ec, :],
                                                  in_=pv1[2 * ec + hh])

                    pv2 = [psC.tile([128, 512], F32, tag="pv", bufs=4,
                                    name=f"pv2_{i}_{pair}_{n}") for n in range(4)]
                    psos = []
                    for j in range(NJ):
                        emit_pv(j, (2, 3), pv2)
                        if j == 0:
                            # finish the per-lane tree (DVE)
                            for hh in range(2):
                                nc.vector.tensor_add(out=sac4[:, hh, :],
                                                     in0=sac3[:, hh, 0, :],
                                                     in1=sac3[:, hh, 1, :])
                        if j == 3:
                            # cross-partition sum via one ones-matmul per
                            # head, then recip + broadcast
                            for hh in range(2):
                                pso = psC.tile([1, 512], F32, tag="ps",
                                               bufs=2,
                                               name=f"pso_{i}_{pair}_{hh}")
                                nc.tensor.matmul(pso, lhsT=ones[:, 0:1],
                                                 rhs=sac4[:, hh, :])
                                psos.append(pso)
                            for hh in range(2):
                                nc.vector.reciprocal_approx_fast(
                                    out=recipS[:, hh, :], in_=psos[hh])
                            recipB = pc1.tile([1, 2, 512], BF16, tag="recipB")
                            nc.vector.tensor_copy(out=recipB, in_=recipS)
                            for hh in range(2):
                                nc.gpsimd.partition_broadcast(
                                    rb[:, hh, :], recipB[:, hh, :])
                        if j == 6:
                            # normalize pass-1 ctx (SBUF -> SBUF)
                            for ec in (0, 1):
                                for hh in range(2):
                                    nc.vector.tensor_mul(
                                        ctxT[h0 + hh][:, ec, :],
                                        ctxU[:, hh, ec, :],
                                        rb[:, hh, :])
                    # pass-2: normalize directly from PSUM
                    for ec in (2, 3):
                        for hh in range(2):
                            nc.vector.tensor_mul(
                                ctxT[h0 + hh][:, ec, :],
                                pv2[2 * (ec % 2) + hh],
                                rb[:, hh, :])

                # --- out-projection for this query chunk ---
                for dm in range(4):
                    for lsub in range(4):
                        ps = psC.tile([128, 512], F32, tag="pv", bufs=4,
                                      name=f"po_{i}_{dm}_{lsub}")
                        for t in range(NDCH):
                            nc.tensor.matmul(
                                ps,
                                lhsT=ctxT[t // 4][:, t % 4,
                                                  lsub * 128:(lsub + 1) * 128],
                                rhs=wo_sb[:, t, dm * 512:(dm + 1) * 512],
                                start=(t == 0), stop=(t == NDCH - 1))
                        ost = pc2.tile([128, 512], BF16, tag="ost")
                        nc.scalar.copy(out=ost, in_=ps)
                        l0 = i * 512 + lsub * 128
                        nc.sync.dma_start(
                            out=out_d.ap()[l0:l0 + 128,
                                           dm * 512:(dm + 1) * 512],
                            in_=ost)

    nc.compile()
    _NC_CACHE["nc"] = nc
    return nc


def make_core_inputs(x, W_attn, W_out):
    """Split full inputs into 8 per-core input maps (core = b*4 + g)."""
    Q_DIM = 2048
    K_DIM = 512
    scale = np.float32(1.0 / math.sqrt(DK))
    bf = ml_dtypes.bfloat16

    # rope tables, mirroring the fp32 reference computation
    inv_freq = (np.float32(1.0) /
                (np.float32(10000.0) **
                 (np.arange(0, DK, 2, dtype=np.float32) / np.float32(DK))))
    freqs = np.arange(L, dtype=np.float32)[:, None] * inv_freq[None, :]  # [L,64]
    ang = np.concatenate([freqs, freqs], axis=-1)  # [L, 128]
    cosT = np.ascontiguousarray(np.cos(ang).T.astype(np.float32))  # [128, L]
    sinT = np.ascontiguousarray(np.sin(ang).T.astype(np.float32))

    def blockT(w):
        # [D_in, C] -> [128, D_in//128, C] so device loads are contiguous
        din, c = w.shape
        return np.ascontiguousarray(
            w.reshape(din // 128, 128, c).transpose(1, 0, 2)).astype(bf)

    in_maps = []
    for core in range(8):
        b, g = divmod(core, 4)
        wq = (W_attn[:, 512 * g:512 * (g + 1)] * scale)
        wk = W_attn[:, Q_DIM + 128 * g:Q_DIM + 128 * (g + 1)]
        wqk = np.concatenate([wq, wk], axis=1)
        wv = W_attn[:, Q_DIM + K_DIM + 512 * g:Q_DIM + K_DIM + 512 * (g + 1)]
        wo = W_out[2048 * g:2048 * (g + 1), :]
        in_maps.append({
            "x": np.ascontiguousarray(x[b]).astype(bf),
            "wqk": blockT(wqk),
            "wv": blockT(wv),
            "wo": blockT(wo),
            "cost": cosT,
            "sint": sinT,
        })
    return in_maps


def kernel(x, W_attn, W_out, b_out, _trace=False, _trace_cores=None):
    x = np.asarray(x)
    W_attn = np.asarray(W_attn)
    W_out = np.asarray(W_out)
    b_out = np.asarray(b_out)
    nc = build_nc()
    in_maps = make_core_inputs(x, W_attn, W_out)
    res = run_bass_kernel_spmd(
        nc, in_maps, core_ids=list(range(8)),
        trace=_trace, trace_cores=_trace_cores)
    parts = [res.results[c]["out"] for c in range(8)]
    out = np.empty((2, L, D), dtype=np.float32)
    for b in range(2):
        acc = parts[4 * b].astype(np.float32)
        for g in range(1, 4):
            acc = acc + parts[4 * b + g].astype(np.float32)
        out[b] = acc + b_out[None, :].astype(np.float32)
    if _trace:
        kernel._last_results = res
    return out


# revision 35
# speedup vs baseline: 1.0582x; 1.0041x over previous
"""GQA kernel for trn2, 8 cores: DP over batch (2) x TP over kv-head groups (4).

Each core computes, for its (batch b, kv-group g):
  - qkv projection for its 4 q-heads + 1 kv-head (q pre-scaled by 1/sqrt(dk))
  - RoPE on q/k
  - full (non-causal) attention for the 4 q-heads vs its kv-head
  - partial out-projection with its 2048 rows of W_out
Host sums the 4 per-group partials per batch and adds bias.

All matmuls bf16 (full PE rate); accumulation fp32. Softmax denominators
are computed off the tensor engine: DVE pairwise adds + GpSimd running
sums + one [128,1]x[128,512] ones-matmul per (i,head), reciprocal via
reciprocal_approx_fast, broadcast via gpsimd. Scores/exp/PV are software
pipelined at key-chunk granularity per head-pair so ACT exp throughput
(~1.1us per [128,1024]) hides under PE matmul streams. PSUM drains are
plain ACT copies; normalization happens in SBUF afterwards so PSUM banks
recycle fast and the PE never waits on the softmax-denominator chain.

Self-contained: hardcodes all shapes. kernel(**inputs) -> np.ndarray.
"""

import math
from contextlib import ExitStack

import numpy as np
import ml_dtypes

import concourse.bass as bass
import concourse.bacc as bacc
import concourse.tile as tile
import concourse.mybir as mybir
from concourse.bass_utils import run_bass_kernel_spmd
from concourse.masks import make_identity

F32 = mybir.dt.float32
BF16 = mybir.dt.bfloat16
L = 2048          # sequence length
D = 2048          # d_model
DK = 128          # head dim (q/k)
DV = 512          # head dim (v)
NHQ = 4           # q heads per core
CQK = NHQ * DK + DK   # 640 qk projection cols per core
NI = 4            # query chunks of 512
NJ = 16           # key chunks of 128
NDCH = 16         # d_model chunks of 128

_NC_CACHE = {}


def build_nc():
    if "nc" in _NC_CACHE:
        return _NC_CACHE["nc"]
    nc = bacc.Bacc("TRN2", target_bir_lowering=False, debug=False)

    # weights arrive pre-rearranged from the host as [p, t, c] blocks so
    # every load is a plain contiguous copy on the hardware DGE
    x_d = nc.dram_tensor("x", [L, D], BF16, kind="ExternalInput")
    wqk_d = nc.dram_tensor("wqk", [128, NDCH, CQK], BF16, kind="ExternalInput")
    wv_d = nc.dram_tensor("wv", [128, NDCH, DV], BF16, kind="ExternalInput")
    wo_d = nc.dram_tensor("wo", [128, NDCH, D], BF16, kind="ExternalInput")
    cos_d = nc.dram_tensor("cost", [DK, L], F32, kind="ExternalInput")
    sin_d = nc.dram_tensor("sint", [DK, L], F32, kind="ExternalInput")
    out_d = nc.dram_tensor("out", [L, D], BF16, kind="ExternalOutput")

    EXP = mybir.ActivationFunctionType.Exp

    with ExitStack() as ctx:
        tc = ctx.enter_context(tile.TileContext(nc))
        persist = ctx.enter_context(tc.tile_pool(name="persist", bufs=1))

        ident = persist.tile([128, 128], BF16)
        make_identity(nc, ident)
        ones = persist.tile([128, 1], BF16)
        nc.vector.memset(ones, 1.0)

        qT = persist.tile([128, NHQ, L], BF16)      # [dk, h, pos]
        kT = persist.tile([128, L], BF16)           # [dk, pos]
        v_sb = persist.tile([128, NJ, DV], BF16)    # [key_in_chunk, key_chunk, e]
        wo_sb = persist.tile([128, NDCH, D], BF16)  # [e_in_chunk, e_chunk, dm]

        # ---------------- Phase B: x^T, qkv projection, rope ----------------
        with tc.tile_pool(name="pb1", bufs=1) as pb1, \
             tc.tile_pool(name="pb2", bufs=2) as pb2, \
             tc.tile_pool(name="psB", bufs=1, space="PSUM") as psB:
            cosT = pb1.tile([128, L], F32)
            sinT = pb1.tile([128, L], F32)
            wv_sb = pb1.tile([128, NDCH, DV], BF16)
            wqk_sb = pb1.tile([128, NDCH, CQK], BF16)

            # first x chunk on the sync queue (latency critical), big
            # prefetches on otherwise-idle engine queues
            xns = {}
            for lsub in range(4):
                xn = pb2.tile([128, D], BF16, tag="xn", bufs=4)
                nc.sync.dma_start(out=xn, in_=x_d.ap()[lsub * 128:(lsub + 1) * 128, :])
                xns[lsub] = xn
            # everything rides the fast hardware DGE in need-order; wv/wo
            # are issued later in program order so they don't block x rows
            nc.sync.dma_start(out=wqk_sb[:, 0:8, :], in_=wqk_d.ap()[:, 0:8, :])
            nc.sync.dma_start(out=wqk_sb[:, 8:16, :], in_=wqk_d.ap()[:, 8:16, :])
            nc.sync.dma_start(out=cosT[:, 0:512], in_=cos_d.ap()[:, 0:512])
            nc.sync.dma_start(out=sinT[:, 0:512], in_=sin_d.ap()[:, 0:512])
            nc.sync.dma_start(out=cosT[:, 512:L], in_=cos_d.ap()[:, 512:L])
            nc.sync.dma_start(out=sinT[:, 512:L], in_=sin_d.ap()[:, 512:L])

            for i in range(NI):
                xT = pb1.tile([128, NDCH, 512], BF16, tag="xT")
                # transpose x rows for this 512-query chunk
                for lsub in range(4):
                    if i > 0:
                        xn = pb2.tile([128, D], BF16, tag="xn", bufs=4)
                        l0 = i * 512 + lsub * 128
                        nc.sync.dma_start(out=xn, in_=x_d.ap()[l0:l0 + 128, :])
                    else:
                        xn = xns[lsub]
                    if i == 0 and lsub == 1:
                        nc.sync.dma_start(out=wv_sb, in_=wv_d.ap())
                    if i == 2 and lsub == 0:
                        nc.sync.dma_start(out=wo_sb, in_=wo_d.ap())
                    for dgrp in range(4):
                        ps = psB.tile([128, 512], BF16, tag="tr", bufs=3)
                        for k in range(4):
                            dch = dgrp * 4 + k
                            nc.tensor.transpose(
                                ps[:, k * 128:(k + 1) * 128],
                                xn[:, dch * 128:(dch + 1) * 128], ident)
                        nc.vector.tensor_copy(
                            out=xT[:, dgrp * 4:dgrp * 4 + 4,
                                   lsub * 128:(lsub + 1) * 128],
                            in_=ps.rearrange("p (a b) -> p a b", a=4))

                # q/k projection + rope (c = 0..3 q heads, c = 4 is k)
                for c in range(5):
                    ps = psB.tile([128, 512], F32, tag="acc", bufs=3)
                    for t in range(NDCH):
                        nc.tensor.matmul(ps, lhsT=wqk_sb[:, t, c * 128:(c + 1) * 128],
                                         rhs=xT[:, t, :],
                                         start=(t == 0), stop=(t == NDCH - 1))
                    isl = slice(i * 512, (i + 1) * 512)
                    dest = qT[:, c, isl] if c < NHQ else kT[:, isl]
                    cs = cosT[:, isl]
                    sn = sinT[:, isl]
                    tmp = pb2.tile([128, 512], F32, tag="rope")
                    nc.vector.tensor_mul(tmp[0:64, :], ps[64:128, :], sn[0:64, :])
                    nc.vector.tensor_mul(tmp[64:128, :], ps[0:64, :], sn[64:128, :])
                    tmp2 = pb2.tile([128, 512], F32, tag="rope2")
                    nc.vector.tensor_mul(tmp2, ps, cs)
                    nc.vector.tensor_sub(dest[0:64, :], tmp2[0:64, :], tmp[0:64, :])
                    nc.vector.tensor_add(dest[64:128, :], tmp2[64:128, :],
                                         tmp[64:128, :])

                # v projection for these 4 key chunks
                for lsub in range(4):
                    ps = psB.tile([128, 512], F32, tag="acc", bufs=3)
                    for t in range(NDCH):
                        nc.tensor.matmul(
                            ps, lhsT=xT[:, t, lsub * 128:(lsub + 1) * 128],
                            rhs=wv_sb[:, t, :],
                            start=(t == 0), stop=(t == NDCH - 1))
                    nc.scalar.copy(out=v_sb[:, i * 4 + lsub, :], in_=ps)

        # ---------------- Phase C+D: attention + out-projection -------------
        with tc.tile_pool(name="pc1", bufs=1) as pc1, \
             tc.tile_pool(name="pc2", bufs=2) as pc2, \
             tc.tile_pool(name="psC", bufs=1, space="PSUM") as psC:
            for i in range(NI):
                isl = slice(i * 512, (i + 1) * 512)
                ctxT = {h: pc1.tile([128, 4, 512], BF16, tag=f"ctx{h}",
                                    name=f"ctxT{i}_{h}")
                        for h in range(NHQ)}

                for pair in range(2):
                    h0 = 2 * pair
                    # expS[keys, hh, j, q] for this head pair
                    expS = pc1.tile([128, 2, NJ, 512], BF16, tag="expS")
                    # denominator reduction tree over key chunks (j):
                    # L1/L2 bf16, L3 bf16, L4 fp32 per-lane totals
                    sacc = pc1.tile([128, 2, 8, 512], BF16, tag="sacc")
                    sac2 = pc1.tile([128, 2, 4, 512], BF16, tag="sac2")
                    sac3 = pc1.tile([128, 2, 2, 512], BF16, tag="sac3")
                    sac4 = pc1.tile([128, 2, 512], BF16, tag="sac4")
                    recipS = pc1.tile([1, 2, 512], F32, tag="recipS")
                    rb = pc1.tile([128, 2, 512], BF16, tag="rb")
                    ctxU = pc1.tile([128, 2, 2, 512], BF16, tag="ctxU")

                    pv1 = [psC.tile([128, 512], F32, tag="pv", bufs=4,
                                    name=f"pv1_{i}_{pair}_{n}") for n in range(4)]

                    def emit_pv(j, ecs, banks):
                        for ec in ecs:
                            for hh in range(2):
                                nc.tensor.matmul(
                                    banks[2 * (ec % 2) + hh],
                                    lhsT=v_sb[:, j, ec * 128:(ec + 1) * 128],
                                    rhs=expS[:, hh, j, :],
                                    start=(j == 0), stop=(j == NJ - 1))

                    def tree_add(dst, src, k, dt_note=None):
                        for hh in range(2):
                            nc.vector.tensor_add(out=dst[:, hh, k, :],
                                                 in0=src[:, hh, 2 * k, :],
                                                 in1=src[:, hh, 2 * k + 1, :])

                    # --- pass 1: scores/exp pipelined with denom + PV ec 0,1
                    for j in range(NJ):
                        jsl = slice(j * 128, (j + 1) * 128)
                        meg = psC.tile([128, 1024], F32, tag="ps", bufs=2,
                                       name=f"meg_{i}_{pair}_{j}")
                        nc.tensor.matmul(meg[:, 0:512], lhsT=kT[:, jsl],
                                         rhs=qT[:, h0, isl])
                        nc.tensor.matmul(meg[:, 512:1024], lhsT=kT[:, jsl],
                                         rhs=qT[:, h0 + 1, isl])
                        nc.scalar.activation(out=expS[:, :, j, :], in_=meg,
                                             func=EXP)
                        if j % 2 == 1:
                            tree_add(sacc, expS, j // 2)
                        if j % 4 == 3:
                            tree_add(sac2, sacc, j // 4)
                        if j % 8 == 7:
                            tree_add(sac3, sac2, j // 8)
                        if j >= 2:
                            emit_pv(j - 2, (0, 1), pv1)
                    emit_pv(NJ - 2, (0, 1), pv1)
                    emit_pv(NJ - 1, (0, 1), pv1)
                    # drain pass-1 banks (DVE copies; normalize later)
                    for ec in (0, 1):
                        for hh in range(2):
                            nc.vector.tensor_copy(out=ctxU[:, hh, ec, :],
                                                  in_=pv1[2 * ec + hh])

                    pv2 = [psC.tile([128, 512], F32, tag="pv", bufs=4,
                                    name=f"pv2_{i}_{pair}_{n}") for n in range(4)]
                    psos = []
                    for j in range(NJ):
                        emit_pv(j, (2, 3), pv2)
                        if j == 0:
                            # finish the per-lane tree (DVE)
                            for hh in range(2):
                                nc.vector.tensor_add(out=sac4[:, hh, :],
                                                     in0=sac3[:, hh, 0, :],
                                                     in1=sac3[:, hh, 1, :])
                        if j == 3:
                            # cross-partition sum via one ones-matmul per
                            # head, then recip + broadcast
                            for hh in range(2):
                                pso = psC.tile([1, 512], F32, tag="ps",
                                               bufs=2,
                                               name=f"pso_{i}_{pair}_{hh}")
                                nc.tensor.matmul(pso, lhsT=ones[:, 0:1],
                                                 rhs=sac4[:, hh, :])
                                psos.append(pso)
                            for hh in range(2):
                                nc.vector.reciprocal_approx_fast(
                                    out=recipS[:, hh, :], in_=psos[hh])
                            recipB = pc1.tile([1, 2, 512], BF16, tag="recipB")
                            nc.vector.tensor_copy(out=recipB, in_=recipS)
                            for hh in range(2):
                                nc.gpsimd.partition_broadcast(
                                    rb[:, hh, :], recipB[:, hh, :])
                        if j == 6:
                            # normalize pass-1 ctx (SBUF -> SBUF)
                            for ec in (0, 1):
                                for hh in range(2):
                                    nc.vector.tensor_mul(
                                        ctxT[h0 + hh][:, ec, :],
                                        ctxU[:, hh, ec, :],
                                        rb[:, hh, :])
                    # pass-2: normalize directly from PSUM
                    for ec in (2, 3):
                        for hh in range(2):
                            nc.vector.tensor_mul(
                                ctxT[h0 + hh][:, ec, :],
                                pv2[2 * (ec % 2) + hh],
                                rb[:, hh, :])

                # --- out-projection for this query chunk ---
                for dm in range(4):
                    for lsub in range(4):
                        ps = psC.tile([128, 512], F32, tag="pv", bufs=4,
                                      name=f"po_{i}_{dm}_{lsub}")
                        for t in range(NDCH):
                            nc.tensor.matmul(
                                ps,
                                lhsT=ctxT[t // 4][:, t % 4,
                                                  lsub * 128:(lsub + 1) * 128],
                                rhs=wo_sb[:, t, dm * 512:(dm + 1) * 512],
                                start=(t == 0), stop=(t == NDCH - 1))
                        ost = pc2.tile([128, 512], BF16, tag="ost")
                        nc.scalar.copy(out=ost, in_=ps)
                        l0 = i * 512 + lsub * 128
                        nc.sync.dma_start(
                            out=out_d.ap()[l0:l0 + 128,
                                           dm * 512:(dm + 1) * 512],
                            in_=ost)

    nc.compile()
    _NC_CACHE["nc"] = nc
    return nc


def make_core_inputs(x, W_attn, W_out):
    """Split full inputs into 8 per-core input maps (core = b*4 + g)."""
    Q_DIM = 2048
    K_DIM = 512
    scale = np.float32(1.0 / math.sqrt(DK))
    bf = ml_dtypes.bfloat16

    # rope tables, mirroring the fp32 reference computation
    inv_freq = (np.float32(1.0) /
                (np.float32(10000.0) **
                 (np.arange(0, DK, 2, dtype=np.float32) / np.float32(DK))))
    freqs = np.arange(L, dtype=np.float32)[:, None] * inv_freq[None, :]  # [L,64]
    ang = np.concatenate([freqs, freqs], axis=-1)  # [L, 128]
    cosT = np.ascontiguousarray(np.cos(ang).T.astype(np.float32))  # [128, L]
    sinT = np.ascontiguousarray(np.sin(ang).T.astype(np.float32))

    def blockT(w):
        # [D_in, C] -> [128, D_in//128, C] so device loads are contiguous
        din, c = w.shape
        return np.ascontiguousarray(
            w.reshape(din // 128, 128, c).transpose(1, 0, 2)).astype(bf)

    in_maps = []
    for core in range(8):
        b, g = divmod(core, 4)
        wq = (W_attn[:, 512 * g:512 * (g + 1)] * scale)
        wk = W_attn[:, Q_DIM + 128 * g:Q_DIM + 128 * (g + 1)]
        wqk = np.concatenate([wq, wk], axis=1)
        wv = W_attn[:, Q_DIM + K_DIM + 512 * g:Q_DIM + K_DIM + 512 * (g + 1)]
        wo = W_out[2048 * g:2048 * (g + 1), :]
        in_maps.append({
            "x": np.ascontiguousarray(x[b]).astype(bf),
            "wqk": blockT(wqk),
            "wv": blockT(wv),
            "wo": blockT(wo),
            "cost": cosT,
            "sint": sinT,
        })
    return in_maps


def kernel(x, W_attn, W_out, b_out, _trace=False, _trace_cores=None):
    x = np.asarray(x)
    W_attn = np.asarray(W_attn)
    W_out = np.asarray(W_out)
    b_out = np.asarray(b_out)
    nc = build_nc()
    in_maps = make_core_inputs(x, W_attn, W_out)
    res = run_bass_kernel_spmd(
        nc, in_maps, core_ids=list(range(8)),
        trace=_trace, trace_cores=_trace_cores)
    parts = [res.results[c]["out"] for c in range(8)]
    out = np.empty((2, L, D), dtype=np.float32)
    for b in range(2):
        acc = parts[4 * b].astype(np.float32)
        for g in range(1, 4):
            acc = acc + parts[4 * b + g].astype(np.float32)
        out[b] = acc + b_out[None, :].astype(np.float32)
    if _trace:
        kernel._last_results = res
    return out


# revision 38
# speedup vs baseline: 1.0604x; 1.0020x over previous
"""GQA kernel for trn2, 8 cores: DP over batch (2) x TP over kv-head groups (4).

Each core computes, for its (batch b, kv-group g):
  - qkv projection for its 4 q-heads + 1 kv-head (q pre-scaled by 1/sqrt(dk))
  - RoPE on q/k
  - full (non-causal) attention for the 4 q-heads vs its kv-head
  - partial out-projection with its 2048 rows of W_out
Host sums the 4 per-group partials per batch and adds bias.

All matmuls bf16 (full PE rate); accumulation fp32. Softmax denominators
are computed off the tensor engine: DVE pairwise adds + GpSimd running
sums + one [128,1]x[128,512] ones-matmul per (i,head), reciprocal via
reciprocal_approx_fast, broadcast via gpsimd. Scores/exp/PV are software
pipelined at key-chunk granularity per head-pair so ACT exp throughput
(~1.1us per [128,1024]) hides under PE matmul streams. PSUM drains are
plain ACT copies; normalization happens in SBUF afterwards so PSUM banks
recycle fast and the PE never waits on the softmax-denominator chain.

Self-contained: hardcodes all shapes. kernel(**inputs) -> np.ndarray.
"""

import math
from contextlib import ExitStack

import numpy as np
import ml_dtypes

import concourse.bass as bass
import concourse.bacc as bacc
import concourse.tile as tile
import concourse.mybir as mybir
from concourse.bass_utils import run_bass_kernel_spmd
from concourse.masks import make_identity

F32 = mybir.dt.float32
BF16 = mybir.dt.bfloat16
L = 2048          # sequence length
D = 2048          # d_model
DK = 128          # head dim (q/k)
DV = 512          # head dim (v)
NHQ = 4           # q heads per core
CQK = NHQ * DK + DK   # 640 qk projection cols per core
NI = 4            # query chunks of 512
NJ = 16           # key chunks of 128
NDCH = 16         # d_model chunks of 128

_NC_CACHE = {}


def build_nc():
    if "nc" in _NC_CACHE:
        return _NC_CACHE["nc"]
    nc = bacc.Bacc("TRN2", target_bir_lowering=False, debug=False)

    # weights arrive pre-rearranged from the host as [p, t, c] blocks so
    # every load is a plain contiguous copy on the hardware DGE
    x_d = nc.dram_tensor("x", [L, D], BF16, kind="ExternalInput")
    wqk_d = nc.dram_tensor("wqk", [128, NDCH, CQK], BF16, kind="ExternalInput")
    wv_d = nc.dram_tensor("wv", [128, NDCH, DV], BF16, kind="ExternalInput")
    wo_d = nc.dram_tensor("wo", [128, NDCH, D], BF16, kind="ExternalInput")
    cos_d = nc.dram_tensor("cost", [DK, L], F32, kind="ExternalInput")
    sin_d = nc.dram_tensor("sint", [DK, L], F32, kind="ExternalInput")
    out_d = nc.dram_tensor("out", [L, D], BF16, kind="ExternalOutput")

    EXP = mybir.ActivationFunctionType.Exp

    with ExitStack() as ctx:
        tc = ctx.enter_context(tile.TileContext(nc))
        persist = ctx.enter_context(tc.tile_pool(name="persist", bufs=1))

        ident = persist.tile([128, 128], BF16)
        make_identity(nc, ident)
        ones = persist.tile([128, 1], BF16)
        nc.vector.memset(ones, 1.0)

        qT = persist.tile([128, NHQ, L], BF16)      # [dk, h, pos]
        kT = persist.tile([128, L], BF16)           # [dk, pos]
        v_sb = persist.tile([128, NJ, DV], BF16)    # [key_in_chunk, key_chunk, e]
        wo_sb = persist.tile([128, NDCH, D], BF16)  # [e_in_chunk, e_chunk, dm]

        # ---------------- Phase B: x^T, qkv projection, rope ----------------
        with tc.tile_pool(name="pb1", bufs=1) as pb1, \
             tc.tile_pool(name="pb2", bufs=2) as pb2, \
             tc.tile_pool(name="psB", bufs=1, space="PSUM") as psB:
            cosT = pb1.tile([128, L], F32)
            sinT = pb1.tile([128, L], F32)
            wv_sb = pb1.tile([128, NDCH, DV], BF16)
            wqk_sb = pb1.tile([128, NDCH, CQK], BF16)

            # first x chunk on the sync queue (latency critical), big
            # prefetches on otherwise-idle engine queues
            xns = {}
            for lsub in range(4):
                xn = pb2.tile([128, D], BF16, tag="xn", bufs=4)
                nc.sync.dma_start(out=xn, in_=x_d.ap()[lsub * 128:(lsub + 1) * 128, :])
                xns[lsub] = xn
            # everything rides the fast hardware DGE in need-order; wv/wo
            # are issued later in program order so they don't block x rows
            nc.sync.dma_start(out=wqk_sb[:, 0:8, :], in_=wqk_d.ap()[:, 0:8, :])
            nc.sync.dma_start(out=wqk_sb[:, 8:16, :], in_=wqk_d.ap()[:, 8:16, :])
            nc.sync.dma_start(out=cosT[:, 0:512], in_=cos_d.ap()[:, 0:512])
            nc.sync.dma_start(out=sinT[:, 0:512], in_=sin_d.ap()[:, 0:512])
            nc.sync.dma_start(out=cosT[:, 512:L], in_=cos_d.ap()[:, 512:L])
            nc.sync.dma_start(out=sinT[:, 512:L], in_=sin_d.ap()[:, 512:L])

            for i in range(NI):
                xT = pb1.tile([128, NDCH, 512], BF16, tag="xT")
                # transpose x rows for this 512-query chunk
                for lsub in range(4):
                    if i > 0:
                        xn = pb2.tile([128, D], BF16, tag="xn", bufs=4)
                        l0 = i * 512 + lsub * 128
                        nc.sync.dma_start(out=xn, in_=x_d.ap()[l0:l0 + 128, :])
                    else:
                        xn = xns[lsub]
                    if i == 0 and lsub == 1:
                        nc.sync.dma_start(out=wv_sb, in_=wv_d.ap())
                    if i == 2 and lsub == 0:
                        nc.sync.dma_start(out=wo_sb, in_=wo_d.ap())
                    for dgrp in range(4):
                        ps = psB.tile([128, 512], BF16, tag="tr", bufs=3)
                        for k in range(4):
                            dch = dgrp * 4 + k
                            nc.tensor.transpose(
                                ps[:, k * 128:(k + 1) * 128],
                                xn[:, dch * 128:(dch + 1) * 128], ident)
                        nc.vector.tensor_copy(
                            out=xT[:, dgrp * 4:dgrp * 4 + 4,
                                   lsub * 128:(lsub + 1) * 128],
                            in_=ps.rearrange("p (a b) -> p a b", a=4))

                # q/k projection + rope (c = 0..3 q heads, c = 4 is k)
                for c in range(5):
                    ps = psB.tile([128, 512], F32, tag="acc", bufs=3)
                    for t in range(NDCH):
                        nc.tensor.matmul(ps, lhsT=wqk_sb[:, t, c * 128:(c + 1) * 128],
                                         rhs=xT[:, t, :],
                                         start=(t == 0), stop=(t == NDCH - 1))
                    isl = slice(i * 512, (i + 1) * 512)
                    dest = qT[:, c, isl] if c < NHQ else kT[:, isl]
                    cs = cosT[:, isl]
                    sn = sinT[:, isl]
                    tmp = pb2.tile([128, 512], F32, tag="rope")
                    nc.vector.tensor_mul(tmp[0:64, :], ps[64:128, :], sn[0:64, :])
                    nc.vector.tensor_mul(tmp[64:128, :], ps[0:64, :], sn[64:128, :])
                    tmp2 = pb2.tile([128, 512], F32, tag="rope2")
                    nc.vector.tensor_mul(tmp2, ps, cs)
                    nc.vector.tensor_sub(dest[0:64, :], tmp2[0:64, :], tmp[0:64, :])
                    nc.vector.tensor_add(dest[64:128, :], tmp2[64:128, :],
                                         tmp[64:128, :])

                # v projection for these 4 key chunks
                for lsub in range(4):
                    ps = psB.tile([128, 512], F32, tag="acc", bufs=3)
                    for t in range(NDCH):
                        nc.tensor.matmul(
                            ps, lhsT=xT[:, t, lsub * 128:(lsub + 1) * 128],
                            rhs=wv_sb[:, t, :],
                            start=(t == 0), stop=(t == NDCH - 1))
                    nc.scalar.copy(out=v_sb[:, i * 4 + lsub, :], in_=ps)

        # ---------------- Phase C+D: attention + out-projection -------------
        with tc.tile_pool(name="pc1", bufs=1) as pc1, \
             tc.tile_pool(name="pc2", bufs=2) as pc2, \
             tc.tile_pool(name="psC", bufs=1, space="PSUM") as psC:
            for i in range(NI):
                isl = slice(i * 512, (i + 1) * 512)
                ctxT = {h: pc1.tile([128, 4, 512], BF16, tag=f"ctx{h}",
                                    name=f"ctxT{i}_{h}")
                        for h in range(NHQ)}

                # shared by both head pairs (region-level deps let the next
                # pair's first scores/exps hide inside this pair's pass 2)
                expS = pc1.tile([128, 2, NJ, 512], BF16, tag="expS")
                sacc = pc1.tile([128, 2, 8, 512], BF16, tag="sacc")
                sac2 = pc1.tile([128, 2, 4, 512], BF16, tag="sac2")
                sac3 = pc1.tile([128, 2, 2, 512], BF16, tag="sac3")
                sac4 = pc1.tile([128, 2, 512], BF16, tag="sac4")

                def emit_sj(pr, j):
                    jsl = slice(j * 128, (j + 1) * 128)
                    meg = psC.tile([128, 1024], F32, tag="ps", bufs=2,
                                   name=f"meg_{i}_{pr}_{j}")
                    nc.tensor.matmul(meg[:, 0:512], lhsT=kT[:, jsl],
                                     rhs=qT[:, 2 * pr, isl])
                    nc.tensor.matmul(meg[:, 512:1024], lhsT=kT[:, jsl],
                                     rhs=qT[:, 2 * pr + 1, isl])
                    nc.scalar.activation(out=expS[:, :, j, :], in_=meg,
                                         func=EXP)

                for pair in range(2):
                    h0 = 2 * pair
                    recipS = pc1.tile([1, 2, 512], F32, tag="recipS")
                    rb = pc1.tile([128, 2, 512], BF16, tag="rb")
                    ctxU = pc1.tile([128, 2, 2, 512], BF16, tag="ctxU")

                    pv1 = [psC.tile([128, 512], F32, tag="pv", bufs=4,
                                    name=f"pv1_{i}_{pair}_{n}") for n in range(4)]

                    def emit_pv(j, ecs, banks):
                        for ec in ecs:
                            for hh in range(2):
                                nc.tensor.matmul(
                                    banks[2 * (ec % 2) + hh],
                                    lhsT=v_sb[:, j, ec * 128:(ec + 1) * 128],
                                    rhs=expS[:, hh, j, :],
                                    start=(j == 0), stop=(j == NJ - 1))

                    def tree_add(dst, src, k, dt_note=None):
                        for hh in range(2):
                            nc.vector.tensor_add(out=dst[:, hh, k, :],
                                                 in0=src[:, hh, 2 * k, :],
                                                 in1=src[:, hh, 2 * k + 1, :])

                    # --- pass 1: scores/exp pipelined with denom + PV ec 0,1
                    for j in range(NJ):
                        if not (pair == 1 and j < 2):
                            emit_sj(pair, j)
                        if j % 2 == 1:
                            tree_add(sacc, expS, j // 2)
                        if j % 4 == 3:
                            tree_add(sac2, sacc, j // 4)
                        if j % 8 == 7:
                            tree_add(sac3, sac2, j // 8)
                        if j >= 2:
                            emit_pv(j - 2, (0, 1), pv1)
                    emit_pv(NJ - 2, (0, 1), pv1)
                    emit_pv(NJ - 1, (0, 1), pv1)
                    # drain pass-1 banks (DVE copies; normalize later)
                    for ec in (0, 1):
                        for hh in range(2):
                            nc.vector.tensor_copy(out=ctxU[:, hh, ec, :],
                                                  in_=pv1[2 * ec + hh])

                    pv2 = [psC.tile([128, 512], F32, tag="pv", bufs=4,
                                    name=f"pv2_{i}_{pair}_{n}") for n in range(4)]
                    psos = []
                    for j in range(NJ):
                        emit_pv(j, (2, 3), pv2)
                        if pair == 0 and j == 8:
                            emit_sj(1, 0)
                        if pair == 0 and j == 10:
                            emit_sj(1, 1)
                        if j == 0:
                            # finish the per-lane tree (DVE)
                            for hh in range(2):
                                nc.vector.tensor_add(out=sac4[:, hh, :],
                                                     in0=sac3[:, hh, 0, :],
                                                     in1=sac3[:, hh, 1, :])
                        if j == 3:
                            # cross-partition sum via one ones-matmul per
                            # head, then recip + broadcast
                            for hh in range(2):
                                pso = psC.tile([1, 512], F32, tag="ps",
                                               bufs=2,
                                               name=f"pso_{i}_{pair}_{hh}")
                                nc.tensor.matmul(pso, lhsT=ones[:, 0:1],
                                                 rhs=sac4[:, hh, :])
                                psos.append(pso)
                            for hh in range(2):
                                nc.vector.reciprocal_approx_fast(
                                    out=recipS[:, hh, :], in_=psos[hh])
                            recipB = pc1.tile([1, 2, 512], BF16, tag="recipB")
                            nc.vector.tensor_copy(out=recipB, in_=recipS)
                            for hh in range(2):
                                nc.gpsimd.partition_broadcast(
                                    rb[:, hh, :], recipB[:, hh, :])
                        if j == 6:
                            # normalize pass-1 ctx (SBUF -> SBUF)
                            for ec in (0, 1):
                                for hh in range(2):
                                    nc.vector.tensor_mul(
                                        ctxT[h0 + hh][:, ec, :],
                                        ctxU[:, hh, ec, :],
                                        rb[:, hh, :])
                    # pass-2: normalize directly from PSUM
                    for ec in (2, 3):
                        for hh in range(2):
                            nc.vector.tensor_mul(
                                ctxT[h0 + hh][:, ec, :],
                                pv2[2 * (ec % 2) + hh],
                                rb[:, hh, :])

                # --- out-projection for this query chunk ---
                for dm in range(4):
                    for lsub in range(4):
                        ps = psC.tile([128, 512], F32, tag="pv", bufs=4,
                                      name=f"po_{i}_{dm}_{lsub}")
                        for t in range(NDCH):
                            nc.tensor.matmul(
                                ps,
                                lhsT=ctxT[t // 4][:, t % 4,
                                                  lsub * 128:(lsub + 1) * 128],
                                rhs=wo_sb[:, t, dm * 512:(dm + 1) * 512],
                                start=(t == 0), stop=(t == NDCH - 1))
                        ost = pc2.tile([128, 512], BF16, tag="ost")
                        nc.scalar.copy(out=ost, in_=ps)
                        l0 = i * 512 + lsub * 128
                        nc.sync.dma_start(
                            out=out_d.ap()[l0:l0 + 128,
                                           dm * 512:(dm + 1) * 512],
                            in_=ost)

    nc.compile()
    _NC_CACHE["nc"] = nc
    return nc


def make_core_inputs(x, W_attn, W_out):
    """Split full inputs into 8 per-core input maps (core = b*4 + g)."""
    Q_DIM = 2048
    K_DIM = 512
    scale = np.float32(1.0 / math.sqrt(DK))
    bf = ml_dtypes.bfloat16

    # rope tables, mirroring the fp32 reference computation
    inv_freq = (np.float32(1.0) /
                (np.float32(10000.0) **
                 (np.arange(0, DK, 2, dtype=np.float32) / np.float32(DK))))
    freqs = np.arange(L, dtype=np.float32)[:, None] * inv_freq[None, :]  # [L,64]
    ang = np.concatenate([freqs, freqs], axis=-1)  # [L, 128]
    cosT = np.ascontiguousarray(np.cos(ang).T.astype(np.float32))  # [128, L]
    sinT = np.ascontiguousarray(np.sin(ang).T.astype(np.float32))

    def blockT(w):
        # [D_in, C] -> [128, D_in//128, C] so device loads are contiguous
        din, c = w.shape
        return np.ascontiguousarray(
            w.reshape(din // 128, 128, c).transpose(1, 0, 2)).astype(bf)

    in_maps = []
    for core in range(8):
        b, g = divmod(core, 4)
        wq = (W_attn[:, 512 * g:512 * (g + 1)] * scale)
        wk = W_attn[:, Q_DIM + 128 * g:Q_DIM + 128 * (g + 1)]
        wqk = np.concatenate([wq, wk], axis=1)
        wv = W_attn[:, Q_DIM + K_DIM + 512 * g:Q_DIM + K_DIM + 512 * (g + 1)]
        wo = W_out[2048 * g:2048 * (g + 1), :]
        in_maps.append({
            "x": np.ascontiguousarray(x[b]).astype(bf),
            "wqk": blockT(wqk),
            "wv": blockT(wv),
            "wo": blockT(wo),
            "cost": cosT,
            "sint": sinT,
        })
    return in_maps


def kernel(x, W_attn, W_out, b_out, _trace=False, _trace_cores=None):
    x = np.asarray(x)
    W_attn = np.asarray(W_attn)
    W_out = np.asarray(W_out)
    b_out = np.asarray(b_out)
    nc = build_nc()
    in_maps = make_core_inputs(x, W_attn, W_out)
    res = run_bass_kernel_spmd(
        nc, in_maps, core_ids=list(range(8)),
        trace=_trace, trace_cores=_trace_cores)
    parts = [res.results[c]["out"] for c in range(8)]
    out = np.empty((2, L, D), dtype=np.float32)
    for b in range(2):
        acc = parts[4 * b].astype(np.float32)
        for g in range(1, 4):
            acc = acc + parts[4 * b + g].astype(np.float32)
        out[b] = acc + b_out[None, :].astype(np.float32)
    if _trace:
        kernel._last_results = res
    return out


# revision 39
# speedup vs baseline: 1.0785x; 1.0171x over previous
"""GQA kernel for trn2, 8 cores: DP over batch (2) x TP over kv-head groups (4).

Each core computes, for its (batch b, kv-group g):
  - qkv projection for its 4 q-heads + 1 kv-head (q pre-scaled by 1/sqrt(dk))
  - RoPE on q/k
  - full (non-causal) attention for the 4 q-heads vs its kv-head
  - partial out-projection with its 2048 rows of W_out
Host sums the 4 per-group partials per batch and adds bias.

All matmuls bf16 (full PE rate); accumulation fp32. Softmax denominators
are computed off the tensor engine: DVE pairwise adds + GpSimd running
sums + one [128,1]x[128,512] ones-matmul per (i,head), reciprocal via
reciprocal_approx_fast, broadcast via gpsimd. Scores/exp/PV are software
pipelined at key-chunk granularity per head-pair so ACT exp throughput
(~1.1us per [128,1024]) hides under PE matmul streams. PSUM drains are
plain ACT copies; normalization happens in SBUF afterwards so PSUM banks
recycle fast and the PE never waits on the softmax-denominator chain.

Self-contained: hardcodes all shapes. kernel(**inputs) -> np.ndarray.
"""

import math
from contextlib import ExitStack

import numpy as np
import ml_dtypes

import concourse.bass as bass
import concourse.bacc as bacc
import concourse.tile as tile
import concourse.mybir as mybir
from concourse.bass_utils import run_bass_kernel_spmd
from concourse.masks import make_identity

F32 = mybir.dt.float32
BF16 = mybir.dt.bfloat16
L = 2048          # sequence length
D = 2048          # d_model
DK = 128          # head dim (q/k)
DV = 512          # head dim (v)
NHQ = 4           # q heads per core
CQK = NHQ * DK + DK   # 640 qk projection cols per core
NI = 4            # query chunks of 512
NJ = 16           # key chunks of 128
NDCH = 16         # d_model chunks of 128

_NC_CACHE = {}


def build_nc():
    if "nc" in _NC_CACHE:
        return _NC_CACHE["nc"]
    nc = bacc.Bacc("TRN2", target_bir_lowering=False, debug=False)

    # weights arrive pre-rearranged from the host as [p, t, c] blocks so
    # every load is a plain contiguous copy on the hardware DGE
    x_d = nc.dram_tensor("x", [L, D], BF16, kind="ExternalInput")
    wqk_d = nc.dram_tensor("wqk", [128, NDCH, CQK], BF16, kind="ExternalInput")
    wv_d = nc.dram_tensor("wv", [128, NDCH, DV], BF16, kind="ExternalInput")
    wo_d = nc.dram_tensor("wo", [128, NDCH, D], BF16, kind="ExternalInput")
    cos_d = nc.dram_tensor("cost", [DK, L], F32, kind="ExternalInput")
    sin_d = nc.dram_tensor("sint", [DK, L], F32, kind="ExternalInput")
    out_d = nc.dram_tensor("out", [L, D], BF16, kind="ExternalOutput")

    EXP = mybir.ActivationFunctionType.Exp

    with ExitStack() as ctx:
        tc = ctx.enter_context(tile.TileContext(nc))
        persist = ctx.enter_context(tc.tile_pool(name="persist", bufs=1))

        ident = persist.tile([128, 128], BF16)
        make_identity(nc, ident)
        ones = persist.tile([128, 1], BF16)
        nc.vector.memset(ones, 1.0)

        qT = persist.tile([128, NHQ, L], BF16)      # [dk, h, pos]
        kT = persist.tile([128, L], BF16)           # [dk, pos]
        v_sb = persist.tile([128, NJ, DV], BF16)    # [key_in_chunk, key_chunk, e]
        wo_sb = persist.tile([128, NDCH, D], BF16)  # [e_in_chunk, e_chunk, dm]

        # ---------------- Phase B: x^T, qkv projection, rope ----------------
        with tc.tile_pool(name="pb1", bufs=1) as pb1, \
             tc.tile_pool(name="pb2", bufs=2) as pb2, \
             tc.tile_pool(name="psB", bufs=1, space="PSUM") as psB:
            cosT = pb1.tile([128, L], F32)
            sinT = pb1.tile([128, L], F32)
            wv_sb = pb1.tile([128, NDCH, DV], BF16)
            wqk_sb = pb1.tile([128, NDCH, CQK], BF16)

            # first x chunk on the sync queue (latency critical), big
            # prefetches on otherwise-idle engine queues
            xns = {}
            for lsub in range(4):
                xn = pb2.tile([128, D], BF16, tag="xn", bufs=4)
                nc.sync.dma_start(out=xn, in_=x_d.ap()[lsub * 128:(lsub + 1) * 128, :])
                xns[lsub] = xn
            # everything rides the fast hardware DGE in need-order; wv/wo
            # are issued later in program order so they don't block x rows
            nc.sync.dma_start(out=wqk_sb[:, 0:8, :], in_=wqk_d.ap()[:, 0:8, :])
            nc.sync.dma_start(out=wqk_sb[:, 8:16, :], in_=wqk_d.ap()[:, 8:16, :])
            nc.sync.dma_start(out=cosT[:, 0:512], in_=cos_d.ap()[:, 0:512])
            nc.sync.dma_start(out=sinT[:, 0:512], in_=sin_d.ap()[:, 0:512])
            nc.sync.dma_start(out=cosT[:, 512:L], in_=cos_d.ap()[:, 512:L])
            nc.sync.dma_start(out=sinT[:, 512:L], in_=sin_d.ap()[:, 512:L])

            for i in range(NI):
                xT = pb1.tile([128, NDCH, 512], BF16, tag="xT")
                # transpose x rows for this 512-query chunk
                for lsub in range(4):
                    if i > 0:
                        xn = pb2.tile([128, D], BF16, tag="xn", bufs=4)
                        l0 = i * 512 + lsub * 128
                        nc.sync.dma_start(out=xn, in_=x_d.ap()[l0:l0 + 128, :])
                    else:
                        xn = xns[lsub]
                    if i == 0 and lsub == 1:
                        nc.sync.dma_start(out=wv_sb, in_=wv_d.ap())
                    if i == 2 and lsub == 0:
                        nc.sync.dma_start(out=wo_sb, in_=wo_d.ap())
                    for dgrp in range(4):
                        ps = psB.tile([128, 512], BF16, tag="tr", bufs=3)
                        for k in range(4):
                            dch = dgrp * 4 + k
                            nc.tensor.transpose(
                                ps[:, k * 128:(k + 1) * 128],
                                xn[:, dch * 128:(dch + 1) * 128], ident)
                        nc.vector.tensor_copy(
                            out=xT[:, dgrp * 4:dgrp * 4 + 4,
                                   lsub * 128:(lsub + 1) * 128],
                            in_=ps.rearrange("p (a b) -> p a b", a=4))

                # q/k projection + rope (c = 0..3 q heads, c = 4 is k)
                for c in range(5):
                    ps = psB.tile([128, 512], F32, tag="acc", bufs=3)
                    for t in range(NDCH):
                        nc.tensor.matmul(ps, lhsT=wqk_sb[:, t, c * 128:(c + 1) * 128],
                                         rhs=xT[:, t, :],
                                         start=(t == 0), stop=(t == NDCH - 1))
                    isl = slice(i * 512, (i + 1) * 512)
                    dest = qT[:, c, isl] if c < NHQ else kT[:, isl]
                    cs = cosT[:, isl]
                    sn = sinT[:, isl]
                    tmp = pb2.tile([128, 512], F32, tag="rope")
                    nc.vector.tensor_mul(tmp[0:64, :], ps[64:128, :], sn[0:64, :])
                    nc.vector.tensor_mul(tmp[64:128, :], ps[0:64, :], sn[64:128, :])
                    tmp2 = pb2.tile([128, 512], F32, tag="rope2")
                    nc.vector.tensor_mul(tmp2, ps, cs)
                    nc.vector.tensor_sub(dest[0:64, :], tmp2[0:64, :], tmp[0:64, :])
                    nc.vector.tensor_add(dest[64:128, :], tmp2[64:128, :],
                                         tmp[64:128, :])

                # v projection for these 4 key chunks
                for lsub in range(4):
                    ps = psB.tile([128, 512], F32, tag="acc", bufs=3)
                    for t in range(NDCH):
                        nc.tensor.matmul(
                            ps, lhsT=xT[:, t, lsub * 128:(lsub + 1) * 128],
                            rhs=wv_sb[:, t, :],
                            start=(t == 0), stop=(t == NDCH - 1))
                    nc.scalar.copy(out=v_sb[:, i * 4 + lsub, :], in_=ps)

        # ---------------- Phase C+D: attention + out-projection -------------
        with tc.tile_pool(name="pc1", bufs=1) as pc1, \
             tc.tile_pool(name="pc2", bufs=2) as pc2, \
             tc.tile_pool(name="psC", bufs=1, space="PSUM") as psC:
            for i in range(NI):
                isl = slice(i * 512, (i + 1) * 512)
                ctxT = {h: pc1.tile([128, 4, 512], BF16, tag=f"ctx{h}",
                                    name=f"ctxT{i}_{h}")
                        for h in range(NHQ)}

                # shared by both head pairs (region-level deps let the next
                # pair's first scores/exps hide inside this pair's pass 2)
                expS = pc1.tile([128, 2, NJ, 512], BF16, tag="expS")
                sacc = pc1.tile([128, 2, 8, 512], BF16, tag="sacc")
                sac2 = pc1.tile([128, 2, 4, 512], BF16, tag="sac2")
                sac3 = pc1.tile([128, 2, 2, 512], BF16, tag="sac3")
                sac4 = pc1.tile([128, 2, 512], BF16, tag="sac4")

                def emit_sj(pr, j):
                    jsl = slice(j * 128, (j + 1) * 128)
                    meg = psC.tile([128, 1024], F32, tag="ps", bufs=2,
                                   name=f"meg_{i}_{pr}_{j}")
                    nc.tensor.matmul(meg[:, 0:512], lhsT=kT[:, jsl],
                                     rhs=qT[:, 2 * pr, isl])
                    nc.tensor.matmul(meg[:, 512:1024], lhsT=kT[:, jsl],
                                     rhs=qT[:, 2 * pr + 1, isl])
                    nc.scalar.activation(out=expS[:, :, j, :], in_=meg,
                                         func=EXP)

                for pair in range(2):
                    h0 = 2 * pair
                    recipS = pc1.tile([1, 2, 512], F32, tag="recipS")
                    rb = pc1.tile([128, 2, 512], BF16, tag="rb")
                    ctxU = pc1.tile([128, 2, 2, 512], BF16, tag="ctxU")

                    pv1 = [psC.tile([128, 512], F32, tag="pv", bufs=4,
                                    name=f"pv1_{i}_{pair}_{n}") for n in range(4)]

                    def emit_pv(j, ecs, banks):
                        for ec in ecs:
                            for hh in range(2):
                                nc.tensor.matmul(
                                    banks[2 * (ec % 2) + hh],
                                    lhsT=v_sb[:, j, ec * 128:(ec + 1) * 128],
                                    rhs=expS[:, hh, j, :],
                                    start=(j == 0), stop=(j == NJ - 1))

                    def tree_add(dst, src, k, dt_note=None):
                        for hh in range(2):
                            nc.vector.tensor_add(out=dst[:, hh, k, :],
                                                 in0=src[:, hh, 2 * k, :],
                                                 in1=src[:, hh, 2 * k + 1, :])

                    # --- pass 1: scores/exp pipelined with denom + PV ec 0,1
                    for j in range(NJ):
                        if not (pair == 1 and j < 2):
                            emit_sj(pair, j)
                        if j % 2 == 1:
                            tree_add(sacc, expS, j // 2)
                        if j % 4 == 3:
                            tree_add(sac2, sacc, j // 4)
                        if j % 8 == 7:
                            tree_add(sac3, sac2, j // 8)
                        if j >= 2:
                            emit_pv(j - 2, (0, 1), pv1)
                    emit_pv(NJ - 2, (0, 1), pv1)
                    emit_pv(NJ - 1, (0, 1), pv1)
                    # drain pass-1 banks on ACT (idle after the last exp) so
                    # neither pass-2 bank recycling nor the DVE tree waits
                    for ec in (0, 1):
                        for hh in range(2):
                            nc.scalar.copy(out=ctxU[:, hh, ec, :],
                                           in_=pv1[2 * ec + hh])

                    pv2 = [psC.tile([128, 512], F32, tag="pv", bufs=4,
                                    name=f"pv2_{i}_{pair}_{n}") for n in range(4)]
                    psos = []
                    for j in range(NJ):
                        emit_pv(j, (2, 3), pv2)
                        if pair == 0 and j == 8:
                            emit_sj(1, 0)
                        if pair == 0 and j == 10:
                            emit_sj(1, 1)
                        if j == 0:
                            # finish the per-lane tree (DVE)
                            for hh in range(2):
                                nc.vector.tensor_add(out=sac4[:, hh, :],
                                                     in0=sac3[:, hh, 0, :],
                                                     in1=sac3[:, hh, 1, :])
                        if j == 3:
                            # cross-partition sum via one ones-matmul per
                            # head, then recip + broadcast
                            for hh in range(2):
                                pso = psC.tile([1, 512], F32, tag="ps",
                                               bufs=2,
                                               name=f"pso_{i}_{pair}_{hh}")
                                nc.tensor.matmul(pso, lhsT=ones[:, 0:1],
                                                 rhs=sac4[:, hh, :])
                                psos.append(pso)
                            for hh in range(2):
                                nc.vector.reciprocal_approx_fast(
                                    out=recipS[:, hh, :], in_=psos[hh])
                            recipB = pc1.tile([1, 2, 512], BF16, tag="recipB")
                            nc.vector.tensor_copy(out=recipB, in_=recipS)
                            for hh in range(2):
                                nc.gpsimd.partition_broadcast(
                                    rb[:, hh, :], recipB[:, hh, :])
                        if j == 6:
                            # normalize pass-1 ctx (SBUF -> SBUF)
                            for ec in (0, 1):
                                for hh in range(2):
                                    nc.vector.tensor_mul(
                                        ctxT[h0 + hh][:, ec, :],
                                        ctxU[:, hh, ec, :],
                                        rb[:, hh, :])
                    # pass-2: normalize directly from PSUM
                    for ec in (2, 3):
                        for hh in range(2):
                            nc.vector.tensor_mul(
                                ctxT[h0 + hh][:, ec, :],
                                pv2[2 * (ec % 2) + hh],
                                rb[:, hh, :])

                # --- out-projection for this query chunk ---
                for dm in range(4):
                    for lsub in range(4):
                        ps = psC.tile([128, 512], F32, tag="pv", bufs=4,
                                      name=f"po_{i}_{dm}_{lsub}")
                        for t in range(NDCH):
                            nc.tensor.matmul(
                                ps,
                                lhsT=ctxT[t // 4][:, t % 4,
                                                  lsub * 128:(lsub + 1) * 128],
                                rhs=wo_sb[:, t, dm * 512:(dm + 1) * 512],
                                start=(t == 0), stop=(t == NDCH - 1))
                        ost = pc2.tile([128, 512], BF16, tag="ost")
                        nc.scalar.copy(out=ost, in_=ps)
                        l0 = i * 512 + lsub * 128
                        nc.sync.dma_start(
                            out=out_d.ap()[l0:l0 + 128,
                                           dm * 512:(dm + 1) * 512],
                            in_=ost)

    nc.compile()
    _NC_CACHE["nc"] = nc
    return nc


def make_core_inputs(x, W_attn, W_out):
    """Split full inputs into 8 per-core input maps (core = b*4 + g)."""
    Q_DIM = 2048
    K_DIM = 512
    scale = np.float32(1.0 / math.sqrt(DK))
    bf = ml_dtypes.bfloat16

    # rope tables, mirroring the fp32 reference computation
    inv_freq = (np.float32(1.0) /
                (np.float32(10000.0) **
                 (np.arange(0, DK, 2, dtype=np.float32) / np.float32(DK))))
    freqs = np.arange(L, dtype=np.float32)[:, None] * inv_freq[None, :]  # [L,64]
    ang = np.concatenate([freqs, freqs], axis=-1)  # [L, 128]
    cosT = np.ascontiguousarray(np.cos(ang).T.astype(np.float32))  # [128, L]
    sinT = np.ascontiguousarray(np.sin(ang).T.astype(np.float32))

    def blockT(w):
        # [D_in, C] -> [128, D_in//128, C] so device loads are contiguous
        din, c = w.shape
        return np.ascontiguousarray(
            w.reshape(din // 128, 128, c).transpose(1, 0, 2)).astype(bf)

    in_maps = []
    for core in range(8):
        b, g = divmod(core, 4)
        wq = (W_attn[:, 512 * g:512 * (g + 1)] * scale)
        wk = W_attn[:, Q_DIM + 128 * g:Q_DIM + 128 * (g + 1)]
        wqk = np.concatenate([wq, wk], axis=1)
        wv = W_attn[:, Q_DIM + K_DIM + 512 * g:Q_DIM + K_DIM + 512 * (g + 1)]
        wo = W_out[2048 * g:2048 * (g + 1), :]
        in_maps.append({
            "x": np.ascontiguousarray(x[b]).astype(bf),
            "wqk": blockT(wqk),
            "wv": blockT(wv),
            "wo": blockT(wo),
            "cost": cosT,
            "sint": sinT,
        })
    return in_maps


def kernel(x, W_attn, W_out, b_out, _trace=False, _trace_cores=None):
    x = np.asarray(x)
    W_attn = np.asarray(W_attn)
    W_out = np.asarray(W_out)
    b_out = np.asarray(b_out)
    nc = build_nc()
    in_maps = make_core_inputs(x, W_attn, W_out)
    res = run_bass_kernel_spmd(
        nc, in_maps, core_ids=list(range(8)),
        trace=_trace, trace_cores=_trace_cores)
    parts = [res.results[c]["out"] for c in range(8)]
    out = np.empty((2, L, D), dtype=np.float32)
    for b in range(2):
        acc = parts[4 * b].astype(np.float32)
        for g in range(1, 4):
            acc = acc + parts[4 * b + g].astype(np.float32)
        out[b] = acc + b_out[None, :].astype(np.float32)
    if _trace:
        kernel._last_results = res
    return out


# revision 43
# speedup vs baseline: 1.1045x; 1.0241x over previous
"""GQA kernel for trn2, 8 cores: DP over batch (2) x TP over kv-head groups (4).

Each core computes, for its (batch b, kv-group g):
  - qkv projection for its 4 q-heads + 1 kv-head (q pre-scaled by 1/sqrt(dk))
  - RoPE on q/k
  - full (non-causal) attention for the 4 q-heads vs its kv-head
  - partial out-projection with its 2048 rows of W_out
Host sums the 4 per-group partials per batch and adds bias.

All matmuls bf16 (full PE rate); accumulation fp32. Softmax denominators
are computed off the tensor engine: DVE pairwise adds + GpSimd running
sums + one [128,1]x[128,512] ones-matmul per (i,head), reciprocal via
reciprocal_approx_fast, broadcast via gpsimd. Scores/exp/PV are software
pipelined at key-chunk granularity per head-pair so ACT exp throughput
(~1.1us per [128,1024]) hides under PE matmul streams. PSUM drains are
plain ACT copies; normalization happens in SBUF afterwards so PSUM banks
recycle fast and the PE never waits on the softmax-denominator chain.

Self-contained: hardcodes all shapes. kernel(**inputs) -> np.ndarray.
"""

import math
from contextlib import ExitStack

import numpy as np
import ml_dtypes

import concourse.bass as bass
import concourse.bacc as bacc
import concourse.tile as tile
import concourse.mybir as mybir
from concourse.bass_utils import run_bass_kernel_spmd
from concourse.masks import make_identity

F32 = mybir.dt.float32
BF16 = mybir.dt.bfloat16
L = 2048          # sequence length
D = 2048          # d_model
DK = 128          # head dim (q/k)
DV = 512          # head dim (v)
NHQ = 4           # q heads per core
CQK = NHQ * DK + DK   # 640 qk projection cols per core
NI = 4            # query chunks of 512
NJ = 16           # key chunks of 128
NDCH = 16         # d_model chunks of 128

_NC_CACHE = {}


def build_nc():
    if "nc" in _NC_CACHE:
        return _NC_CACHE["nc"]
    nc = bacc.Bacc("TRN2", target_bir_lowering=False, debug=False)

    # weights arrive pre-rearranged from the host as [p, t, c] blocks so
    # every load is a plain contiguous copy on the hardware DGE
    xt_d = nc.dram_tensor("xt", [128, NDCH, L], BF16, kind="ExternalInput")
    wqk_d = nc.dram_tensor("wqk", [128, NDCH, CQK], BF16, kind="ExternalInput")
    wv_d = nc.dram_tensor("wv", [128, NDCH, DV], BF16, kind="ExternalInput")
    wo_d = nc.dram_tensor("wo", [128, NDCH, D], BF16, kind="ExternalInput")
    cos_d = nc.dram_tensor("cost", [DK, L], F32, kind="ExternalInput")
    sin_d = nc.dram_tensor("sint", [DK, L], F32, kind="ExternalInput")
    out_d = nc.dram_tensor("out", [L, D], BF16, kind="ExternalOutput")

    EXP = mybir.ActivationFunctionType.Exp

    with ExitStack() as ctx:
        tc = ctx.enter_context(tile.TileContext(nc))
        persist = ctx.enter_context(tc.tile_pool(name="persist", bufs=1))

        ones = persist.tile([128, 1], BF16)
        nc.vector.memset(ones, 1.0)

        qT = persist.tile([128, NHQ, L], BF16)      # [dk, h, pos]
        kT = persist.tile([128, L], BF16)           # [dk, pos]
        v_sb = persist.tile([128, NJ, DV], BF16)    # [key_in_chunk, key_chunk, e]
        wo_sb = persist.tile([128, NDCH, D], BF16)  # [e_in_chunk, e_chunk, dm]

        # ---------------- Phase B: x^T, qkv projection, rope ----------------
        with tc.tile_pool(name="pb1", bufs=1) as pb1, \
             tc.tile_pool(name="pb2", bufs=2) as pb2, \
             tc.tile_pool(name="psB", bufs=1, space="PSUM") as psB:
            cosT = pb1.tile([128, L], F32)
            sinT = pb1.tile([128, L], F32)
            wv_sb = pb1.tile([128, NDCH, DV], BF16)
            wqk_sb = pb1.tile([128, NDCH, CQK], BF16)

            # x arrives pre-transposed from the host ([dk, d_chunk, pos]);
            # everything rides the fast hardware DGE in need-order, wv/wo
            # issued later in program order so they don't block the front
            xTs = {}
            xTs[0] = pb1.tile([128, NDCH, 512], BF16, tag="xT", bufs=2,
                              name="xT0")
            nc.sync.dma_start(out=xTs[0], in_=xt_d.ap()[:, :, 0:512])
            nc.sync.dma_start(out=wqk_sb[:, 0:8, :], in_=wqk_d.ap()[:, 0:8, :])
            nc.sync.dma_start(out=wqk_sb[:, 8:16, :], in_=wqk_d.ap()[:, 8:16, :])
            nc.sync.dma_start(out=cosT[:, 0:512], in_=cos_d.ap()[:, 0:512])
            nc.sync.dma_start(out=sinT[:, 0:512], in_=sin_d.ap()[:, 0:512])
            nc.sync.dma_start(out=cosT[:, 512:L], in_=cos_d.ap()[:, 512:L])
            nc.sync.dma_start(out=sinT[:, 512:L], in_=sin_d.ap()[:, 512:L])
            nc.sync.dma_start(out=wv_sb, in_=wv_d.ap())

            for i in range(NI):
                xT = xTs[i]
                if i + 1 < NI:
                    xTs[i + 1] = pb1.tile([128, NDCH, 512], BF16, tag="xT",
                                          bufs=2, name=f"xT{i + 1}")
                    nc.sync.dma_start(
                        out=xTs[i + 1],
                        in_=xt_d.ap()[:, :, (i + 1) * 512:(i + 2) * 512])
                if i == 2:
                    nc.sync.dma_start(out=wo_sb, in_=wo_d.ap())

                # q/k projection + rope (c = 0..3 q heads, c = 4 is k)
                for c in range(5):
                    ps = psB.tile([128, 512], F32, tag="acc", bufs=3)
                    for t in range(NDCH):
                        nc.tensor.matmul(ps, lhsT=wqk_sb[:, t, c * 128:(c + 1) * 128],
                                         rhs=xT[:, t, :],
                                         start=(t == 0), stop=(t == NDCH - 1))
                    isl = slice(i * 512, (i + 1) * 512)
                    dest = qT[:, c, isl] if c < NHQ else kT[:, isl]
                    cs = cosT[:, isl]
                    sn = sinT[:, isl]
                    tmp = pb2.tile([128, 512], F32, tag="rope")
                    nc.vector.tensor_mul(tmp[0:64, :], ps[64:128, :], sn[0:64, :])
                    nc.vector.tensor_mul(tmp[64:128, :], ps[0:64, :], sn[64:128, :])
                    tmp2 = pb2.tile([128, 512], F32, tag="rope2")
                    nc.vector.tensor_mul(tmp2, ps, cs)
                    nc.vector.tensor_sub(dest[0:64, :], tmp2[0:64, :], tmp[0:64, :])
                    nc.vector.tensor_add(dest[64:128, :], tmp2[64:128, :],
                                         tmp[64:128, :])

                # v projection for these 4 key chunks
                for lsub in range(4):
                    ps = psB.tile([128, 512], F32, tag="acc", bufs=3)
                    for t in range(NDCH):
                        nc.tensor.matmul(
                            ps, lhsT=xT[:, t, lsub * 128:(lsub + 1) * 128],
                            rhs=wv_sb[:, t, :],
                            start=(t == 0), stop=(t == NDCH - 1))
                    nc.scalar.copy(out=v_sb[:, i * 4 + lsub, :], in_=ps)

        # ---------------- Phase C+D: attention + out-projection -------------
        with tc.tile_pool(name="pc1", bufs=1) as pc1, \
             tc.tile_pool(name="pc2", bufs=2) as pc2, \
             tc.tile_pool(name="psC", bufs=1, space="PSUM") as psC:
            for i in range(NI):
                isl = slice(i * 512, (i + 1) * 512)
                ctxT = {h: pc1.tile([128, 4, 512], BF16, tag=f"ctx{h}",
                                    name=f"ctxT{i}_{h}")
                        for h in range(NHQ)}

                # shared by both head pairs (region-level deps let the next
                # pair's first scores/exps hide inside this pair's pass 2)
                expS = pc1.tile([128, 2, NJ, 512], BF16, tag="expS")
                sacc = pc1.tile([128, 2, 8, 512], BF16, tag="sacc")
                sac2 = pc1.tile([128, 2, 4, 512], BF16, tag="sac2")
                sac3 = pc1.tile([128, 2, 2, 512], BF16, tag="sac3")
                sac4 = pc1.tile([128, 2, 512], BF16, tag="sac4")

                def emit_sj(pr, j):
                    jsl = slice(j * 128, (j + 1) * 128)
                    meg = psC.tile([128, 1024], F32, tag="ps", bufs=2,
                                   name=f"meg_{i}_{pr}_{j}")
                    nc.tensor.matmul(meg[:, 0:512], lhsT=kT[:, jsl],
                                     rhs=qT[:, 2 * pr, isl])
                    nc.tensor.matmul(meg[:, 512:1024], lhsT=kT[:, jsl],
                                     rhs=qT[:, 2 * pr + 1, isl])
                    nc.scalar.activation(out=expS[:, :, j, :], in_=meg,
                                         func=EXP)

                for pair in range(2):
                    h0 = 2 * pair
                    recipS = pc1.tile([1, 2, 512], F32, tag="recipS")
                    rb = pc1.tile([128, 2, 512], BF16, tag="rb")
                    ctxU = pc1.tile([128, 2, 2, 512], BF16, tag="ctxU")

                    pv1 = [psC.tile([128, 512], F32, tag="pv", bufs=4,
                                    name=f"pv1_{i}_{pair}_{n}") for n in range(4)]

                    def emit_pv(j, ecs, banks):
                        for ec in ecs:
                            for hh in range(2):
                                nc.tensor.matmul(
                                    banks[2 * (ec % 2) + hh],
                                    lhsT=v_sb[:, j, ec * 128:(ec + 1) * 128],
                                    rhs=expS[:, hh, j, :],
                                    start=(j == 0), stop=(j == NJ - 1))

                    def tree_add(dst, src, k, dt_note=None):
                        for hh in range(2):
                            nc.vector.tensor_add(out=dst[:, hh, k, :],
                                                 in0=src[:, hh, 2 * k, :],
                                                 in1=src[:, hh, 2 * k + 1, :])

                    # --- pass 1: scores/exp pipelined with denom + PV ec 0,1
                    for j in range(NJ):
                        if not (pair == 1 and j < 2):
                            emit_sj(pair, j)
                        if j % 2 == 1:
                            tree_add(sacc, expS, j // 2)
                        if j % 4 == 3:
                            tree_add(sac2, sacc, j // 4)
                        if j % 8 == 7:
                            tree_add(sac3, sac2, j // 8)
                        if j >= 2:
                            emit_pv(j - 2, (0, 1), pv1)
                    emit_pv(NJ - 2, (0, 1), pv1)
                    emit_pv(NJ - 1, (0, 1), pv1)
                    # drain pass-1 banks on ACT (idle after the last exp) so
                    # neither pass-2 bank recycling nor the DVE tree waits
                    for ec in (0, 1):
                        for hh in range(2):
                            nc.scalar.copy(out=ctxU[:, hh, ec, :],
                                           in_=pv1[2 * ec + hh])

                    pv2 = [psC.tile([128, 512], F32, tag="pv", bufs=4,
                                    name=f"pv2_{i}_{pair}_{n}") for n in range(4)]
                    psos = []
                    for j in range(NJ):
                        emit_pv(j, (2, 3), pv2)
                        if pair == 0 and j == 8:
                            emit_sj(1, 0)
                        if pair == 0 and j == 10:
                            emit_sj(1, 1)
                        if j == 0:
                            # finish the per-lane tree (DVE)
                            for hh in range(2):
                                nc.vector.tensor_add(out=sac4[:, hh, :],
                                                     in0=sac3[:, hh, 0, :],
                                                     in1=sac3[:, hh, 1, :])
                        if j == 3:
                            # cross-partition sum via one ones-matmul per
                            # head, then recip + broadcast
                            for hh in range(2):
                                pso = psC.tile([1, 512], F32, tag="ps",
                                               bufs=2,
                                               name=f"pso_{i}_{pair}_{hh}")
                                nc.tensor.matmul(pso, lhsT=ones[:, 0:1],
                                                 rhs=sac4[:, hh, :])
                                psos.append(pso)
                            for hh in range(2):
                                nc.vector.reciprocal_approx_fast(
                                    out=recipS[:, hh, :], in_=psos[hh])
                            recipB = pc1.tile([1, 2, 512], BF16, tag="recipB")
                            nc.vector.tensor_copy(out=recipB, in_=recipS)
                            for hh in range(2):
                                nc.gpsimd.partition_broadcast(
                                    rb[:, hh, :], recipB[:, hh, :])
                        if j == 6:
                            # normalize pass-1 ctx (SBUF -> SBUF)
                            for ec in (0, 1):
                                for hh in range(2):
                                    nc.vector.tensor_mul(
                                        ctxT[h0 + hh][:, ec, :],
                                        ctxU[:, hh, ec, :],
                                        rb[:, hh, :])
                    # pass-2: normalize directly from PSUM
                    for ec in (2, 3):
                        for hh in range(2):
                            nc.vector.tensor_mul(
                                ctxT[h0 + hh][:, ec, :],
                                pv2[2 * (ec % 2) + hh],
                                rb[:, hh, :])

                # --- out-projection for this query chunk ---
                for dm in range(4):
                    for lsub in range(4):
                        ps = psC.tile([128, 512], F32, tag="pv", bufs=4,
                                      name=f"po_{i}_{dm}_{lsub}")
                        for t in range(NDCH):
                            nc.tensor.matmul(
                                ps,
                                lhsT=ctxT[t // 4][:, t % 4,
                                                  lsub * 128:(lsub + 1) * 128],
                                rhs=wo_sb[:, t, dm * 512:(dm + 1) * 512],
                                start=(t == 0), stop=(t == NDCH - 1))
                        ost = pc2.tile([128, 512], BF16, tag="ost")
                        nc.scalar.copy(out=ost, in_=ps)
                        l0 = i * 512 + lsub * 128
                        nc.sync.dma_start(
                            out=out_d.ap()[l0:l0 + 128,
                                           dm * 512:(dm + 1) * 512],
                            in_=ost)

    nc.compile()
    _NC_CACHE["nc"] = nc
    return nc


def make_core_inputs(x, W_attn, W_out):
    """Split full inputs into 8 per-core input maps (core = b*4 + g)."""
    Q_DIM = 2048
    K_DIM = 512
    scale = np.float32(1.0 / math.sqrt(DK))
    bf = ml_dtypes.bfloat16

    # rope tables, mirroring the fp32 reference computation
    inv_freq = (np.float32(1.0) /
                (np.float32(10000.0) **
                 (np.arange(0, DK, 2, dtype=np.float32) / np.float32(DK))))
    freqs = np.arange(L, dtype=np.float32)[:, None] * inv_freq[None, :]  # [L,64]
    ang = np.concatenate([freqs, freqs], axis=-1)  # [L, 128]
    cosT = np.ascontiguousarray(np.cos(ang).T.astype(np.float32))  # [128, L]
    sinT = np.ascontiguousarray(np.sin(ang).T.astype(np.float32))

    def blockT(w):
        # [D_in, C] -> [128, D_in//128, C] so device loads are contiguous
        din, c = w.shape
        return np.ascontiguousarray(
            w.reshape(din // 128, 128, c).transpose(1, 0, 2)).astype(bf)

    in_maps = []
    for core in range(8):
        b, g = divmod(core, 4)
        wq = (W_attn[:, 512 * g:512 * (g + 1)] * scale)
        wk = W_attn[:, Q_DIM + 128 * g:Q_DIM + 128 * (g + 1)]
        wqk = np.concatenate([wq, wk], axis=1)
        wv = W_attn[:, Q_DIM + K_DIM + 512 * g:Q_DIM + K_DIM + 512 * (g + 1)]
        wo = W_out[2048 * g:2048 * (g + 1), :]
        xt = np.ascontiguousarray(
            x[b].T.reshape(NDCH, 128, L).transpose(1, 0, 2)).astype(bf)
        in_maps.append({
            "xt": xt,
            "wqk": blockT(wqk),
            "wv": blockT(wv),
            "wo": blockT(wo),
            "cost": cosT,
            "sint": sinT,
        })
    return in_maps


def kernel(x, W_attn, W_out, b_out, _trace=False, _trace_cores=None):
    x = np.asarray(x)
    W_attn = np.asarray(W_attn)
    W_out = np.asarray(W_out)
    b_out = np.asarray(b_out)
    nc = build_nc()
    in_maps = make_core_inputs(x, W_attn, W_out)
    res = run_bass_kernel_spmd(
        nc, in_maps, core_ids=list(range(8)),
        trace=_trace, trace_cores=_trace_cores)
    parts = [res.results[c]["out"] for c in range(8)]
    out = np.empty((2, L, D), dtype=np.float32)
    for b in range(2):
        acc = parts[4 * b].astype(np.float32)
        for g in range(1, 4):
            acc = acc + parts[4 * b + g].astype(np.float32)
        out[b] = acc + b_out[None, :].astype(np.float32)
    if _trace:
        kernel._last_results = res
    return out
